# revision 2
# baseline (speedup 1.0000x reference)
"""Trainium2 Bass kernel for nn_CogAgentDecoderLayer (8-core SPMD).

Feature-major activations [feat, tok] in permuted token order
(vis-only | both | lang-only | neither). TP: QKV/dense by heads (2/core),
MLP by intermediate slice (688/core), cross-attn token-parallel
(256 tok/core). Device collectives: ReduceScatter after self-attn dense,
AllGather of normed h2 before MLP; final partial-sum reduce on host.
Self-attention runs in permuted order with a host-built causal mask.
bf16 for h/qkv/attn-probs/cross-kv, fp32r for dense/cq/cdense/MLP.
"""
import os
import numpy as np
from contextlib import ExitStack
from concourse import bacc, tile, mybir
from concourse.bass_utils import run_bass_kernel_spmd

NC_ = 8
S, E, H, NH, HD = 2048, 2048, 2048, 16, 128
CH, CC, CHD = 1024, 1024, 64
I = 5504
IS = I // NC_          # 688
ISP = 768              # padded to 6*128
EPS = 1e-5
ROPE_BASE = 10000.0
F32 = mybir.dt.float32
F32R = mybir.dt.float32r
BF16 = mybir.dt.bfloat16
DVE_F32R = True        # DVE may write fp32r tiles directly


def _segs(lo, hi, b0, b1, b2):
    pts = sorted({lo, hi, *[b for b in (b0, b1, b2) if lo < b < hi]})
    out = []
    for s, e in zip(pts, pts[1:]):
        ex = []
        if s < b1:
            ex.append(0)
        if b0 <= s < b2:
            ex.append(1)
        out.append((s, e, ex))
    return out


def _chunks(lo, hi, w):
    out = []
    while lo < hi:
        out.append((lo, min(lo + w, hi)))
        lo += w
    return out


def build_kernel(b0, b1, b2):
    nc = bacc.Bacc("TRN2", target_bir_lowering=False, debug=False,
                   num_devices=NC_)
    din = lambda n, sh, dt: nc.dram_tensor(n, sh, dt, kind="ExternalInput")
    hT = din("hT", [H, S], BF16)
    wqkv0 = din("wqkv0", [H, 768], BF16)
    wqkv1 = din("wqkv1", [H, 768], BF16)
    wd0 = din("wd0", [256, H], F32R)
    wd1 = din("wd1", [256, H], F32R)
    cos2 = din("cos2", [128, S], BF16)
    sin2 = din("sin2", [128, S], BF16)
    rotT = din("rotT", [128, 128], BF16)
    onesr = din("onesr", [128, 128], F32R)
    onesb = din("onesb", [128, 128], BF16)
    zeros = din("zeros", [128, 512], F32R)
    maskneg = din("maskneg", [S, S], BF16)
    resid = din("resid", [H, 256], F32R)
    encT = din("encT", [CH, E], BF16)
    wk = din("wk", [CH, CC], BF16)
    wvv = din("wvv", [CH, CC], BF16)
    wcq = din("wcq", [H, CC], F32R)
    wcd = din("wcd", [CC, H], F32R)
    wgu0 = din("wgu0", [H, 2 * IS], BF16)
    wgu1 = din("wgu1", [H, 2 * IS], BF16)
    wdn0 = din("wdn0", [ISP, H], BF16)
    wdn1 = din("wdn1", [ISP, H], BF16)
    y = nc.dram_tensor("y", [H, S], F32, kind="ExternalOutput")

    SC = 1.0 / float(np.sqrt(HD))
    CSC = 1.0 / float(np.sqrt(CHD))
    EXP = mybir.ActivationFunctionType.Exp
    SQ = mybir.ActivationFunctionType.Square
    SQRT = mybir.ActivationFunctionType.Sqrt
    SILU = mybir.ActivationFunctionType.Silu
    r128 = lambda ap: ap.rearrange("(c p) n -> p c n", p=128)

    with tile.TileContext(nc) as tc, ExitStack() as top:
        const = top.enter_context(tc.tile_pool(name="const", bufs=1))
        ones_sb = const.tile([128, 128], F32R)
        nc.sync.dma_start(ones_sb[:], onesr.ap()[:])
        ones_bf = const.tile([128, 128], BF16)
        nc.sync.dma_start(ones_bf[:], onesb.ap()[:])
        rot_sb = const.tile([128, 128], BF16)
        nc.sync.dma_start(rot_sb[:], rotT.ap()[:])
        from concourse.masks import make_identity
        ident = const.tile([128, 128], BF16)
        make_identity(nc, ident[:])
        cos_sb = const.tile([128, S], BF16)
        nc.sync.dma_start(cos_sb[:], cos2.ap()[:])
        sin_sb = const.tile([128, S], BF16)
        nc.sync.dma_start(sin_sb[:], sin2.ap()[:])
        zer_sb = const.tile([128, 512], F32R)
        nc.sync.dma_start(zer_sb[:], zeros.ap()[:])
        eps_sb = const.tile([128, 1], F32)
        nc.vector.memset(eps_sb[:], EPS)

        dram = top.enter_context(tc.tile_pool(name="dram", bufs=1, space="DRAM"))
        bounce = dram.tile([NC_ * H, 256], F32)
        rs_out = dram.tile([H, 256], F32)
        h2n_bnc = dram.tile([H, 256], BF16)
        h2n_all = dram.tile([NC_ * H, 256], BF16, addr_space="Shared")
        h2out = nc.dram_tensor("h2out", [H, 256], F32, kind="ExternalOutput")

        scrp = top.enter_context(tc.tile_pool(name="scr", bufs=2))

        def vwrite(op, dst, a, bb):
            if DVE_F32R:
                op(dst, a, bb)
            else:
                scr = scrp.tile([dst.shape[0], dst.shape[-1]], F32,
                                name="vscr", tag="vscr")
                op(scr[:], a, bb)
                nc.scalar.copy(dst, scr[:])

        pABC = top.enter_context(ExitStack())
        qkp = pABC.enter_context(tc.tile_pool(name="qkp", bufs=1))
        qkv_sb = qkp.tile([128, 6, S], BF16)      # q0 q1 k0 k1 v0 v1
        v_sb = qkp.tile([128, 16, 256], BF16)     # token-major v
        ctxp = pABC.enter_context(tc.tile_pool(name="ctxp", bufs=1))
        ctx_sb = ctxp.tile([128, 2, S], F32R)

        # ===== phase A: h load + rmsnorm + QKV + rope + vT =====
        with ExitStack() as pA:
            hp = pA.enter_context(tc.tile_pool(name="hp", bufs=1))
            h_sb = hp.tile([128, 16, S], BF16)
            nc.sync.dma_start(h_sb[:], r128(hT.ap()))
            with ExitStack() as pA1:
                nrm = pA1.enter_context(tc.tile_pool(name="nrm", bufs=2))
                nps = pA1.enter_context(tc.tile_pool(name="nps", bufs=2,
                                                     space="PSUM"))
                for t0, t1 in _chunks(0, S, 512):
                    pss = nps.tile([128, 512], F32, name="pss", tag="pss")
                    for kc in range(16):
                        sq = nrm.tile([128, 512], F32R, name="sq", tag="sq")
                        nc.scalar.activation(sq[:], h_sb[:, kc, t0:t1], SQ)
                        nc.tensor.matmul(pss[:], ones_sb[:], sq[:],
                                         start=(kc == 0), stop=(kc == 15))
                    rms = nrm.tile([128, 512], F32, name="rms", tag="rms")
                    nc.scalar.activation(rms[:], pss[:], SQRT,
                                         scale=1.0 / H, bias=eps_sb[:])
                    rinv = nrm.tile([128, 512], F32, name="rinv", tag="rinv")
                    nc.vector.reciprocal(rinv[:], rms[:])
                    for kc in range(16):
                        nc.vector.tensor_mul(h_sb[:, kc, t0:t1],
                                             h_sb[:, kc, t0:t1], rinv[:])
            with ExitStack() as pA2:
                wp = pA2.enter_context(tc.tile_pool(name="wp", bufs=3))
                mps = pA2.enter_context(tc.tile_pool(name="mps", bufs=2,
                                                     space="PSUM"))
                for slot in range(6):
                    wts = []
                    for ex, wsrc in ((0, wqkv0), (1, wqkv1)):
                        wt = wp.tile([128, 16, 128], BF16,
                                     name=f"wq{ex}{slot}", tag=f"wq{ex}")
                        nc.sync.dma_start(
                            wt[:], r128(wsrc.ap()[:, slot * 128:slot * 128 + 128]))
                        wts.append(wt)
                    for t0, t1 in _chunks(0, S, 512):
                        sg = [x for x in _segs(t0, t1, b0, b1, b2) if x[2]]
                        if not sg:
                            continue
                        need = sorted({x for _, _, ex in sg for x in ex})
                        pss_ = {}
                        for x in need:
                            ps = mps.tile([128, 512], F32, name=f"qps{x}",
                                          tag=f"qps{x}")
                            for kc in range(16):
                                nc.tensor.matmul(ps[:], wts[x][:, kc, :],
                                                 h_sb[:, kc, t0:t1],
                                                 start=(kc == 0), stop=(kc == 15))
                            pss_[x] = ps
                        for s, e, ex in sg:
                            if len(ex) == 1:
                                nc.vector.tensor_copy(qkv_sb[:, slot, s:e],
                                                      pss_[ex[0]][:, s - t0:e - t0])
                            else:
                                nc.vector.tensor_add(qkv_sb[:, slot, s:e],
                                                     pss_[0][:, s - t0:e - t0],
                                                     pss_[1][:, s - t0:e - t0])
                    if b2 < S:
                        nc.vector.memset(qkv_sb[:, slot, b2:S], 0.0)
                # rope on q,k
                for slot in range(4):
                    for t0, t1 in _chunks(0, S, 512):
                        rp = mps.tile([128, 512], F32, name="rps", tag="qps")
                        nc.tensor.matmul(rp[:], rot_sb[:],
                                         qkv_sb[:, slot, t0:t1],
                                         start=True, stop=True)
                        c1 = scrp.tile([128, 512], F32, name="ropec", tag="ropec")
                        nc.vector.tensor_mul(c1[:], qkv_sb[:, slot, t0:t1],
                                             cos_sb[:, t0:t1])
                        s1 = scrp.tile([128, 512], F32, name="ropes", tag="ropes")
                        nc.vector.tensor_mul(s1[:], rp[:], sin_sb[:, t0:t1])
                        nc.vector.tensor_add(qkv_sb[:, slot, t0:t1],
                                             c1[:], s1[:])
                # v -> token-major via PE transpose
                for hh in range(2):
                    for tt in range(16):
                        tp = mps.tile([128, 512], BF16, name="tps", tag="qps")
                        nc.tensor.transpose(
                            tp[:, :128],
                            qkv_sb[:, 4 + hh, tt * 128:tt * 128 + 128],
                            ident[:])
                        nc.vector.tensor_copy(v_sb[:, tt, hh * 128:hh * 128 + 128],
                                       tp[:, :128])
        # ===== phase B: self-attention (perm order) =====
        with ExitStack() as pB:
            ap_ = pB.enter_context(tc.tile_pool(name="ap", bufs=3))
            aps = pB.enter_context(tc.tile_pool(name="aps", bufs=2, space="PSUM"))
            accp = pB.enter_context(tc.tile_pool(name="accp", bufs=1, space="PSUM"))
            for t0, t1 in _chunks(0, S, 512):
                pss_ = [accp.tile([128, 512], F32, name=f"pssum{h}", tag=f"pssum{h}")
                        for h in range(2)]
                psc_ = [accp.tile([128, 512], F32, name=f"psctx{h}", tag=f"psctx{h}")
                        for h in range(2)]
                for kt in range(16):
                    mt_ = ap_.tile([128, 512], BF16, name="mt", tag="mt")
                    nc.sync.dma_start(
                        mt_[:], maskneg.ap()[kt * 128:kt * 128 + 128, t0:t1])
                    for hh in range(2):
                        sc = aps.tile([128, 512], F32, name="sc", tag="sc")
                        nc.tensor.matmul(
                            sc[:], qkv_sb[:, 2 + hh, kt * 128:kt * 128 + 128],
                            qkv_sb[:, hh, t0:t1], start=True, stop=True)
                        nc.vector.tensor_add(sc[:], sc[:], mt_[:])
                        pr = ap_.tile([128, 512], BF16, name="pr", tag="pr")
                        nc.scalar.activation(pr[:], sc[:], EXP, scale=SC)
                        nc.tensor.matmul(pss_[hh][:], ones_bf[:], pr[:],
                                         start=(kt == 0), stop=(kt == 15))
                        nc.tensor.matmul(
                            psc_[hh][:], v_sb[:, kt, hh * 128:hh * 128 + 128],
                            pr[:], start=(kt == 0), stop=(kt == 15))
                for hh in range(2):
                    rc = ap_.tile([128, 512], F32, name="rc", tag="rc")
                    nc.vector.reciprocal(rc[:], pss_[hh][:])
                    vwrite(nc.vector.tensor_mul, ctx_sb[:, hh, t0:t1],
                           psc_[hh][:], rc[:])
        # ===== phase C: dense (routed) -> bounce -> RS =====
        with ExitStack() as pC:
            dwp = pC.enter_context(tc.tile_pool(name="dwp", bufs=1))
            dps = pC.enter_context(tc.tile_pool(name="dps", bufs=2, space="PSUM"))
            dop = pC.enter_context(tc.tile_pool(name="dop", bufs=4))
            dwts = []
            for ex, wsrc in ((0, wd0), (1, wd1)):
                dwt = dwp.tile([128, 2, H], F32R, name=f"dw{ex}", tag=f"dw{ex}")
                nc.sync.dma_start(dwt[:], r128(wsrc.ap()))
                dwts.append(dwt)
            for tt in range(8):
                t0, t1 = tt * 256, tt * 256 + 256
                sg = _segs(t0, t1, b0, b1, b2)
                live = [x for x in sg if x[2]]
                for mt in range(16):
                    ot = dop.tile([128, 256], F32, name="dot", tag="dot")
                    if live:
                        need = sorted({x for _, _, ex in live for x in ex})
                        pss_ = {}
                        for x in need:
                            ps = dps.tile([128, 256], F32, name=f"dpst{x}",
                                          tag=f"dpst{x}")
                            for kc in range(2):
                                nc.tensor.matmul(
                                    ps[:],
                                    dwts[x][:, kc, mt * 128:mt * 128 + 128],
                                    ctx_sb[:, kc, t0:t1],
                                    start=(kc == 0), stop=(kc == 1))
                            pss_[x] = ps
                        for s, e, ex in sg:
                            if len(ex) == 2:
                                nc.vector.tensor_add(ot[:, s - t0:e - t0],
                                                     pss_[0][:, s - t0:e - t0],
                                                     pss_[1][:, s - t0:e - t0])
                            elif ex:
                                nc.vector.tensor_copy(ot[:, s - t0:e - t0],
                                                      pss_[ex[0]][:, s - t0:e - t0])
                            else:
                                nc.vector.memset(ot[:, s - t0:e - t0], 0.0)
                    else:
                        nc.vector.memset(ot[:], 0.0)
                    nc.sync.dma_start(
                        bounce[tt * H + mt * 128: tt * H + mt * 128 + 128, :],
                        ot[:])
        pABC.close()
        nc.gpsimd.collective_compute(
            "ReduceScatter", mybir.AluOpType.add,
            replica_groups=[list(range(NC_))],
            ins=[bounce.opt()], outs=[rs_out.opt()])

        # ===== phase D: cross attention (token-parallel) =====
        with ExitStack() as pD:
            dp = pD.enter_context(tc.tile_pool(name="dp", bufs=1))
            dps2 = pD.enter_context(tc.tile_pool(name="dps2", bufs=2, space="PSUM"))
            h1_sb = dp.tile([128, 16, 256], F32R)
            cq_sb = dp.tile([128, 8, 256], BF16)
            cctx_sb = dp.tile([128, 8, 256], F32R)
            with ExitStack() as pD1:
                d1 = pD1.enter_context(tc.tile_pool(name="d1", bufs=1))
                rs_sb = d1.tile([128, 16, 256], F32)
                nc.sync.dma_start(rs_sb[:], r128(rs_out[:]))
                re_sb = d1.tile([128, 16, 256], F32R)
                nc.sync.dma_start(re_sb[:], r128(resid.ap()))
                for kc in range(16):
                    vwrite(nc.vector.tensor_add, h1_sb[:, kc, :],
                           rs_sb[:, kc, :], re_sb[:, kc, :].bitcast(F32))
                pss = dps2.tile([128, 256], F32, name="psd", tag="psd")
                for kc in range(16):
                    sq = scrp.tile([128, 256], F32R, name="sqd", tag="sqd")
                    nc.scalar.activation(sq[:], h1_sb[:, kc, :].bitcast(F32), SQ)
                    nc.tensor.matmul(pss[:], ones_sb[:], sq[:],
                                     start=(kc == 0), stop=(kc == 15))
                rms = scrp.tile([128, 256], F32, name="rmsd", tag="rmsd")
                nc.scalar.activation(rms[:], pss[:], SQRT, scale=1.0 / H, bias=eps_sb[:])
                rinv = d1.tile([128, 256], F32)
                nc.vector.reciprocal(rinv[:], rms[:])
                h1n_sb = d1.tile([128, 16, 256], F32R)
                for kc in range(16):
                    vwrite(nc.vector.tensor_mul, h1n_sb[:, kc, :],
                           h1_sb[:, kc, :].bitcast(F32), rinv[:])
                for mt in range(8):
                    wcq_t = d1.tile([128, 16, 128], F32R, name="wcqt", tag="wcqt",
                                    bufs=2)
                    nc.sync.dma_start(
                        wcq_t[:], r128(wcq.ap()[:, mt * 128:mt * 128 + 128]))
                    ps = dps2.tile([128, 256], F32, name="cqp", tag="psd")
                    for kc in range(16):
                        nc.tensor.matmul(ps[:],
                                         wcq_t[:, kc, :],
                                         h1n_sb[:, kc, :],
                                         start=(kc == 0), stop=(kc == 15))
                    nc.vector.tensor_copy(cq_sb[:, mt, :], ps[:])
            with ExitStack() as pD2:
                kp = pD2.enter_context(tc.tile_pool(name="kp", bufs=1))
                k_sb = kp.tile([128, 8, E], BF16)
                v_sb2 = kp.tile([128, 16, CC], BF16)
                with ExitStack() as pD2e:
                    ep = pD2e.enter_context(tc.tile_pool(name="ep", bufs=1))
                    enc_sb = ep.tile([128, 8, E], BF16)
                    nc.sync.dma_start(enc_sb[:], r128(encT.ap()))
                    wk_sb = ep.tile([128, 8, CC], BF16)
                    nc.sync.dma_start(wk_sb[:], r128(wk.ap()))
                    wv_sb = ep.tile([128, 8, CC], BF16)
                    nc.sync.dma_start(wv_sb[:], r128(wvv.ap()))
                    for mt in range(8):
                        for n0, n1 in _chunks(0, E, 512):
                            ps = dps2.tile([128, 512], F32, name="kps", tag="kps")
                            for kc in range(8):
                                nc.tensor.matmul(
                                    ps[:], wk_sb[:, kc, mt * 128:mt * 128 + 128],
                                    enc_sb[:, kc, n0:n1],
                                    start=(kc == 0), stop=(kc == 7))
                            nc.vector.tensor_copy(k_sb[:, mt, n0:n1], ps[:])
                    for tt in range(16):
                        for n0, n1 in _chunks(0, CC, 512):
                            ps = dps2.tile([128, 512], F32, name="vps", tag="kps")
                            for kc in range(8):
                                nc.tensor.matmul(
                                    ps[:], enc_sb[:, kc, tt * 128:tt * 128 + 128],
                                    wv_sb[:, kc, n0:n1],
                                    start=(kc == 0), stop=(kc == 7))
                            nc.vector.tensor_copy(v_sb2[:, tt, n0:n1], ps[:])
                with ExitStack() as pD3:
                    cap = pD3.enter_context(tc.tile_pool(name="cap", bufs=3))
                    caps = pD3.enter_context(tc.tile_pool(name="caps", bufs=2,
                                                          space="PSUM"))
                    cacc = pD3.enter_context(tc.tile_pool(name="cacc", bufs=1,
                                                          space="PSUM"))
                    for h in range(NH):
                        kch, koff = h // 2, 64 * (h % 2)
                        pssum = cacc.tile([128, 256], F32, name="cps", tag="cps")
                        psctx = cacc.tile([64, 256], F32, name="cpc", tag="cpc")
                        for kt in range(16):
                            sc = caps.tile([128, 256], F32, name="csc", tag="csc")
                            nc.tensor.matmul(
                                sc[:],
                                k_sb[koff:koff + 64, kch, kt * 128:kt * 128 + 128],
                                cq_sb[koff:koff + 64, kch, :],
                                start=True, stop=True)
                            pr = cap.tile([128, 256], BF16, name="cpr", tag="cpr")
                            nc.scalar.activation(pr[:], sc[:], EXP, scale=CSC)
                            nc.tensor.matmul(pssum[:], ones_bf[:], pr[:],
                                             start=(kt == 0), stop=(kt == 15))
                            nc.tensor.matmul(psctx[:],
                                             v_sb2[:, kt, 64 * h:64 * h + 64],
                                             pr[:], start=(kt == 0), stop=(kt == 15))
                        rc = cap.tile([64, 256], F32, name="crc", tag="crc")
                        nc.vector.reciprocal(rc[:], pssum[:64, :])
                        vwrite(nc.vector.tensor_mul,
                               cctx_sb[koff:koff + 64, kch, :], psctx[:], rc[:])
            # cdense + residual -> h2, rmsnorm -> h2n -> AG
            with ExitStack() as pD4:
                d4 = pD4.enter_context(tc.tile_pool(name="d4", bufs=1))
                h2_sb = d4.tile([128, 16, 256], F32)
                h2n_sb = d4.tile([128, 16, 256], BF16)
                wcd_sb = d4.tile([128, 8, H], F32R)
                nc.sync.dma_start(wcd_sb[:], r128(wcd.ap()))
                for mt in range(16):
                    ps = dps2.tile([128, 256], F32, name="cdp", tag="psd")
                    for kc in range(8):
                        nc.tensor.matmul(ps[:],
                                         wcd_sb[:, kc, mt * 128:mt * 128 + 128],
                                         cctx_sb[:, kc, :],
                                         start=(kc == 0), stop=(kc == 7))
                    nc.vector.tensor_add(h2_sb[:, mt, :], ps[:],
                                         h1_sb[:, mt, :].bitcast(F32))
                pss2 = dps2.tile([128, 256], F32, name="psd2", tag="psd")
                for kc in range(16):
                    sq = scrp.tile([128, 256], F32R, name="sqd2", tag="sqd")
                    nc.scalar.activation(sq[:], h2_sb[:, kc, :], SQ)
                    nc.tensor.matmul(pss2[:], ones_sb[:], sq[:],
                                     start=(kc == 0), stop=(kc == 15))
                rms2 = scrp.tile([128, 256], F32, name="rmsd2", tag="rmsd")
                nc.scalar.activation(rms2[:], pss2[:], SQRT,
                                     scale=1.0 / H, bias=eps_sb[:])
                rinv2 = d4.tile([128, 256], F32)
                nc.vector.reciprocal(rinv2[:], rms2[:])
                for kc in range(16):
                    nc.vector.tensor_mul(h2n_sb[:, kc, :],
                                         h2_sb[:, kc, :], rinv2[:])
                nc.sync.dma_start(r128(h2n_bnc[:]), h2n_sb[:])
                nc.sync.dma_start(r128(h2out.ap()), h2_sb[:])
            nc.gpsimd.collective_compute(
                "AllGather", mybir.AluOpType.bypass,
                replica_groups=[list(range(NC_))],
                ins=[h2n_bnc.opt()], outs=[h2n_all.opt()])
        # ===== phase F: MLP (routed by expert ranges, bf16) =====
        with ExitStack() as pF:
            fp = pF.enter_context(tc.tile_pool(name="fp", bufs=1))
            hn_sb = fp.tile([128, 16, S], BF16)
            for r in range(NC_):
                nc.sync.dma_start(hn_sb[:, :, r * 256:r * 256 + 256],
                                  r128(h2n_all[r * H:(r + 1) * H, :]))
            fw = pF.enter_context(tc.tile_pool(name="fw", bufs=1))
            fps = pF.enter_context(tc.tile_pool(name="fps", bufs=1, space="PSUM"))
            fpd = pF.enter_context(tc.tile_pool(name="fpd", bufs=2, space="PSUM"))
            fac = pF.enter_context(tc.tile_pool(name="fac", bufs=2))
            fout = pF.enter_context(tc.tile_pool(name="fout", bufs=4))
            for ex, (lo, hi) in ((0, (0, b1)), (1, (b1, S))):
                gsrc = (wgu0, wgu1)[ex]
                dsrc = (wdn0, wdn1)[ex]
                dn_t = fw.tile([128, 6, H], BF16, name=f"dn{ex}", tag="dn")
                nc.sync.dma_start(dn_t[:], r128(dsrc.ap()))
                gwts = []
                for pi in range(6):
                    gw = 128 if pi < 5 else 48
                    gwt = fw.tile([128, 16, 256], BF16,
                                  name=f"guw{ex}{pi}", tag=f"guw{pi}")
                    nc.sync.dma_start(
                        gwt[:, :, :2 * gw],
                        r128(gsrc.ap()[:, pi * 256:pi * 256 + 2 * gw]))
                    gwts.append(gwt)
                for a0 in range(0, S, 512):
                    c0, c1 = max(a0, lo), min(a0 + 512, hi)
                    if c0 >= c1:
                        continue
                    t0_, W = a0, 512
                    eo, ew = c0 - a0, c1 - c0
                    act = fac.tile([128, 6, 512], BF16, name="act", tag="act")
                    for pi in range(6):
                        gw = 128 if pi < 5 else 48
                        gwt = gwts[pi]
                        pg = fps.tile([128, 512], F32, name="pg", tag="pg")
                        pu = fps.tile([128, 512], F32, name="pu", tag="pu")
                        for kc in range(16):
                            nc.tensor.matmul(pg[:gw, :W], gwt[:, kc, :gw],
                                             hn_sb[:, kc, t0_:t0_ + 512],
                                             start=(kc == 0), stop=(kc == 15))
                            nc.tensor.matmul(pu[:gw, :W], gwt[:, kc, gw:2 * gw],
                                             hn_sb[:, kc, t0_:t0_ + 512],
                                             start=(kc == 0), stop=(kc == 15))
                        gs = scrp.tile([128, 512], F32, name="gs", tag="gs")
                        nc.scalar.activation(gs[:gw, :W], pg[:gw, :W], SILU)
                        nc.vector.tensor_mul(act[:gw, pi, :W],
                                             gs[:gw, :W], pu[:gw, :W])
                    for mt in range(16):
                        pd = fpd.tile([128, 512], F32, name="pd", tag="pd")
                        for pi in range(6):
                            kw = 128 if pi < 5 else 48
                            nc.tensor.matmul(
                                pd[:, :W],
                                dn_t[:kw, pi, mt * 128:mt * 128 + 128],
                                act[:kw, pi, :W],
                                start=(pi == 0), stop=(pi == 5))
                        ot = fout.tile([128, 512], F32, name="fot", tag="fot")
                        nc.vector.tensor_copy(ot[:, eo:eo + ew], pd[:, eo:eo + ew])
                        nc.sync.dma_start(
                            y.ap()[mt * 128:mt * 128 + 128, c0:c1],
                            ot[:, eo:eo + ew])
    nc.compile()
    return nc


_CACHE = {}


def kernel(**inputs):
    import ml_dtypes
    vm = np.asarray(inputs["vision_token_ids"]).astype(bool)
    lm = np.asarray(inputs["language_token_ids"]).astype(bool)
    g0 = np.where(vm & ~lm)[0]; g1 = np.where(vm & lm)[0]
    g2 = np.where(~vm & lm)[0]; g3 = np.where(~vm & ~lm)[0]
    perm = np.concatenate([g0, g1, g2, g3])
    b0 = len(g0); b1 = b0 + len(g1); b2 = b1 + len(g2)

    f32 = lambda x: np.ascontiguousarray(np.asarray(x, np.float32))
    bf = lambda x: np.ascontiguousarray(np.asarray(x).astype(ml_dtypes.bfloat16))
    pos = np.asarray(inputs["positions"]).astype(np.float32)
    half = HD // 2
    inv_freq = 1.0 / (ROPE_BASE ** (np.arange(half, dtype=np.float32) / half))
    fr = pos[:, None] * inv_freq[None, :]
    cos2 = np.concatenate([np.cos(fr)] * 2, 1).T[:, perm]
    sin2 = np.concatenate([np.sin(fr)] * 2, 1).T[:, perm]
    rot = np.zeros((HD, HD), np.float32)
    rot[np.arange(half), np.arange(half) + half] = -1.0
    rot[np.arange(half) + half, np.arange(half)] = 1.0
    op = np.asarray(inputs["positions"])[perm]
    maskneg = np.where(op[None, :] >= op[:, None], 0.0, -30000.0)

    wln_in = f32(inputs["w_ln_in"])[:, None]
    wln_pa = f32(inputs["w_ln_post_attn"])[:, None]
    wln_pc = f32(inputs["w_ln_post_cross"])[:, None]
    wqkv = [f32(inputs["w_vis_qkv"]) * wln_in, f32(inputs["w_lang_qkv"]) * wln_in]
    wd = [f32(inputs["w_vis_dense"]), f32(inputs["w_lang_dense"])]
    wgu = [f32(inputs["w_vis_gate_up"]) * wln_pc,
           f32(inputs["w_lang_gate_up"]) * wln_pc]
    wdn = [f32(inputs["w_vis_down"]), f32(inputs["w_lang_down"])]
    wkvf = f32(inputs["w_cross_kv"])
    hTp = f32(inputs["hidden_states"]).T[:, perm].copy()

    def interleave(w):  # w [H, 2*IS] = [gate | up]
        cols = []
        for i in range(5):
            cols.append(w[:, 128 * i:128 * i + 128])
            cols.append(w[:, IS + 128 * i:IS + 128 * i + 128])
        cols.append(w[:, 640:IS]); cols.append(w[:, IS + 640:2 * IS])
        return np.ascontiguousarray(np.concatenate(cols, 1))

    key = (b0, b1, b2)
    if key not in _CACHE:
        _CACHE.clear()
        _CACHE[key] = build_kernel(b0, b1, b2)
    nc = _CACHE[key]

    in_maps = []
    for c in range(NC_):
        qs = slice(256 * c, 256 * c + 256)
        m = dict(
            hT=bf(hTp),
            wqkv0=bf(np.concatenate([wqkv[0][:, qs], wqkv[0][:, H:][:, qs],
                                     wqkv[0][:, 2 * H:][:, qs]], 1)),
            wqkv1=bf(np.concatenate([wqkv[1][:, qs], wqkv[1][:, H:][:, qs],
                                     wqkv[1][:, 2 * H:][:, qs]], 1)),
            wd0=wd[0][qs].copy(), wd1=wd[1][qs].copy(),
            cos2=bf(cos2), sin2=bf(sin2), rotT=bf(rot.T),
            onesr=np.ones((128, 128), np.float32),
            onesb=np.ones((128, 128), ml_dtypes.bfloat16),
            zeros=np.zeros((128, 512), np.float32),
            maskneg=bf(maskneg), resid=hTp[:, qs].copy(),
            encT=bf(f32(inputs["encoder_embeds"]).T),
            wk=bf(wkvf[:, :CC]), wvv=bf(wkvf[:, CC:]),
            wcq=(f32(inputs["w_cross_q"]) * wln_pa).copy(),
            wcd=f32(inputs["w_cross_dense"]),
            wgu0=bf(interleave(np.concatenate(
                [wgu[0][:, IS * c:IS * c + IS],
                 wgu[0][:, I + IS * c:I + IS * c + IS]], 1))),
            wgu1=bf(interleave(np.concatenate(
                [wgu[1][:, IS * c:IS * c + IS],
                 wgu[1][:, I + IS * c:I + IS * c + IS]], 1))),
            wdn0=bf(np.concatenate([wdn[0][IS * c:IS * c + IS],
                                    np.zeros((ISP - IS, H), np.float32)], 0)),
            wdn1=bf(np.concatenate([wdn[1][IS * c:IS * c + IS],
                                    np.zeros((ISP - IS, H), np.float32)], 0)),
        )
        in_maps.append(m)

    trace = bool(int(os.environ.get("KTRACE", "0")))
    res = run_bass_kernel_spmd(nc, in_maps, core_ids=list(range(NC_)),
                               trace=trace,
                               tmpdir=os.environ.get("KTRACE_DIR") or None)
    kernel.last_exec_ns = res.exec_time_ns
    kernel.last_trace = res.instructions_and_trace
    tot = res.results[0]["y"].astype(np.float64)
    for c in range(1, NC_):
        tot += res.results[c]["y"]
    for c in range(NC_):
        tot[:, 256 * c:256 * c + 256] += res.results[c]["h2out"]
    out = np.empty((S, H), np.float32)
    out[perm, :] = tot.T.astype(np.float32)
    return out



# revision 25
# speedup vs baseline: 1.2784x; 1.2784x over previous
"""Trainium2 Bass kernel for nn_CogAgentDecoderLayer (8-core SPMD).

Feature-major activations [feat, tok] in permuted token order
(vis-only | both | lang-only | neither). TP plan per core c
(hh = c//4 head-half, g = c%4 token-group):
  - self-attn: heads split 2/core, causal block-skip, masks only for
    mixed diagonal blocks; dense row-parallel bf16 -> feature-split
    ReduceScatter x2 (RS_a overlaps dense 2nd half).
  - RS token slot T_c = [512*g + 256*hh, +256).
  - cross-attn: KV proj E-sharded + 4-group AllGather (issued early,
    hides behind self-attn); per-core 8 heads x 512 tokens (N=512
    matmuls); softmax denom folded into AV via 65th ones-row; cdense
    half-contraction + pairwise ReduceScatter.
  - MLP: I sliced 688/core, routed by expert; h2n AllGather split by
    feature halves so AG_b hides behind first-half gate_up. y bf16
    partials summed on host.
All matmuls bf16 (fp32 matmul is 2x rows on PE).
"""
import os
import numpy as np
from contextlib import ExitStack
from concourse import bacc, tile, mybir
from concourse.bass_utils import run_bass_kernel_spmd

NC_ = 8
S, E, H, NH, HD = 2048, 2048, 2048, 16, 128
CH, CC, CHD = 1024, 1024, 64
I = 5504
IS = I // NC_          # 688
ISP = 768              # padded to 6*128
EPS = 1e-5
ROPE_BASE = 10000.0
F32 = mybir.dt.float32
BF16 = mybir.dt.bfloat16


def _segs(lo, hi, b0, b1, b2):
    pts = sorted({lo, hi, *[b for b in (b0, b1, b2) if lo < b < hi]})
    out = []
    for s, e in zip(pts, pts[1:]):
        ex = []
        if s < b1:
            ex.append(0)
        if b0 <= s < b2:
            ex.append(1)
        out.append((s, e, ex))
    return out


def _chunks(lo, hi, w):
    out = []
    while lo < hi:
        out.append((lo, min(lo + w, hi)))
        lo += w
    return out


def build_kernel(b0, b1, b2, blocks):
    # blocks: dict (qc, kt) -> ('F', -1) or ('M', mix_idx); absent = skip
    n_mix = max(1, sum(1 for v in blocks.values() if v[0] == 'M'))
    nc = bacc.Bacc("TRN2", target_bir_lowering=False, debug=False,
                   num_devices=NC_)
    din = lambda n, sh, dt: nc.dram_tensor(n, sh, dt, kind="ExternalInput")
    hT = din("hT", [H, S], BF16)
    wqkv0 = din("wqkv0", [H, 768], BF16)
    wqkv1 = din("wqkv1", [H, 768], BF16)
    wd0 = din("wd0", [256, H], BF16)
    wd1 = din("wd1", [256, H], BF16)
    cos2 = din("cos2", [128, S], BF16)
    sin2 = din("sin2", [128, S], BF16)
    rotT = din("rotT", [128, 128], BF16)
    onesb = din("onesb", [128, 128], BF16)
    maskmix = din("maskmix", [n_mix * 128, 512], BF16)
    resid = din("resid", [H, 256], F32)
    encTs = din("encTs", [CH, 512], BF16)
    wkh = din("wkh", [CH, 512], BF16)
    wvh = din("wvh", [CH, 512], BF16)
    wcqh = din("wcqh", [H, 512], BF16)
    wcdh = din("wcdh", [512, H], BF16)
    wgu0 = din("wgu0", [H, 2 * IS], BF16)
    wgu1 = din("wgu1", [H, 2 * IS], BF16)
    wdn0 = din("wdn0", [ISP, H], BF16)
    wdn1 = din("wdn1", [ISP, H], BF16)
    y = nc.dram_tensor("y", [H, S], BF16, kind="ExternalOutput")

    SC = 1.0 / float(np.sqrt(HD))
    CSC = 1.0 / float(np.sqrt(CHD))
    EXP = mybir.ActivationFunctionType.Exp
    SQ = mybir.ActivationFunctionType.Square
    SQRT = mybir.ActivationFunctionType.Sqrt
    SILU = mybir.ActivationFunctionType.Silu
    r128 = lambda ap: ap.rearrange("(c p) n -> p c n", p=128)

    with tile.TileContext(nc) as tc, ExitStack() as top:
        const = top.enter_context(tc.tile_pool(name="const", bufs=1))
        ones_bf = const.tile([128, 128], BF16)
        nc.sync.dma_start(ones_bf[:], onesb.ap()[:])
        rot_sb = const.tile([128, 128], BF16)
        nc.sync.dma_start(rot_sb[:], rotT.ap()[:])
        from concourse.masks import make_identity
        ident = const.tile([128, 128], BF16)
        make_identity(nc, ident[:])
        cos_sb = const.tile([128, S], BF16)
        nc.sync.dma_start(cos_sb[:], cos2.ap()[:])
        sin_sb = const.tile([128, S], BF16)
        nc.sync.dma_start(sin_sb[:], sin2.ap()[:])
        eps_sb = const.tile([128, 1], F32)
        nc.vector.memset(eps_sb[:], EPS)

        dram = top.enter_context(tc.tile_pool(name="dram", bufs=1, space="DRAM"))
        bounce_a = dram.tile([8 * 1024, 256], BF16)
        bounce_b = dram.tile([8 * 1024, 256], BF16)
        rs_a = dram.tile([1024, 256], BF16)
        rs_b = dram.tile([1024, 256], BF16)
        h1n_bnc = dram.tile([H, 256], BF16)
        h1n_pair = dram.tile([2 * H, 256], BF16)
        kvcon = dram.tile([1024, 520], BF16)
        kvagg = dram.tile([4096, 520], BF16)
        cdpart = dram.tile([2 * H, 256], BF16)
        cdsum = dram.tile([H, 256], BF16)
        h2n_bnc_a = dram.tile([1024, 256], BF16)
        h2n_bnc_b = dram.tile([1024, 256], BF16)
        h2na = dram.tile([8 * 1024, 256], BF16, addr_space="Shared")
        h2nb = dram.tile([8 * 1024, 256], BF16, addr_space="Shared")
        h2out = nc.dram_tensor("h2out", [H, 256], F32, kind="ExternalOutput")

        scrp = top.enter_context(tc.tile_pool(name="scr", bufs=2))

        pABC = top.enter_context(ExitStack())
        qkp = pABC.enter_context(tc.tile_pool(name="qkp", bufs=1))
        qkv_sb = qkp.tile([128, 6, S], BF16)      # q0 q1 k0 k1 v0 v1
        v_sb = qkp.tile([128, 16, 256], BF16)     # token-major v
        ctxp = pABC.enter_context(tc.tile_pool(name="ctxp", bufs=1))
        ctx_sb = ctxp.tile([128, 2, S], BF16)

        # ===== phase A: h load + rmsnorm + QKV + rope + vT =====
        with ExitStack() as pA:
            hp = pA.enter_context(tc.tile_pool(name="hp", bufs=1))
            h_sb = [hp.tile([128, 16, 512], BF16, name=f"h{i}")
                    for i in range(4)]
            with ExitStack() as pA1:
                nrm = pA1.enter_context(tc.tile_pool(name="nrm", bufs=2))
                nps = pA1.enter_context(tc.tile_pool(name="nps", bufs=2,
                                                     space="PSUM"))
                for ci, (t0, t1) in enumerate(_chunks(0, S, 512)):
                    nc.sync.dma_start(h_sb[ci][:], r128(hT.ap()[:, t0:t1]))
                    pss = nps.tile([128, 512], F32, name="pss", tag="pss")
                    for kc in range(16):
                        sq = nrm.tile([128, 512], BF16, name="sq", tag="sq")
                        nc.scalar.activation(sq[:], h_sb[ci][:, kc, :], SQ)
                        nc.tensor.matmul(pss[:], ones_bf[:], sq[:],
                                         start=(kc == 0), stop=(kc == 15))
                    rms = nrm.tile([128, 512], F32, name="rms", tag="rms")
                    nc.scalar.activation(rms[:], pss[:], SQRT,
                                         scale=1.0 / H, bias=eps_sb[:])
                    rinv = nrm.tile([128, 512], F32, name="rinv", tag="rinv")
                    nc.vector.reciprocal(rinv[:], rms[:])
                    for kc in range(16):
                        nc.vector.tensor_mul(h_sb[ci][:, kc, :],
                                             h_sb[ci][:, kc, :], rinv[:])
            with ExitStack() as pA2:
                wp = pA2.enter_context(tc.tile_pool(name="wp", bufs=3))
                mps = pA2.enter_context(tc.tile_pool(name="mps", bufs=2,
                                                     space="PSUM"))
                for slot in range(6):
                    wts = []
                    for ex, wsrc in ((0, wqkv0), (1, wqkv1)):
                        wt = wp.tile([128, 16, 128], BF16,
                                     name=f"wq{ex}{slot}", tag=f"wq{ex}")
                        nc.sync.dma_start(
                            wt[:], r128(wsrc.ap()[:, slot * 128:slot * 128 + 128]))
                        wts.append(wt)
                    for ci, (t0, t1) in enumerate(_chunks(0, S, 512)):
                        sg = [x for x in _segs(t0, t1, b0, b1, b2) if x[2]]
                        if not sg:
                            continue
                        need = sorted({x for _, _, ex in sg for x in ex})
                        pss_ = {}
                        for x in need:
                            ps = mps.tile([128, 512], F32, name=f"qps{x}",
                                          tag=f"qps{x}")
                            for kc in range(16):
                                nc.tensor.matmul(ps[:], wts[x][:, kc, :],
                                                 h_sb[ci][:, kc, :],
                                                 start=(kc == 0), stop=(kc == 15))
                            pss_[x] = ps
                        for s, e, ex in sg:
                            if len(ex) == 1:
                                nc.vector.tensor_copy(qkv_sb[:, slot, s:e],
                                                      pss_[ex[0]][:, s - t0:e - t0])
                            else:
                                nc.vector.tensor_add(qkv_sb[:, slot, s:e],
                                                     pss_[0][:, s - t0:e - t0],
                                                     pss_[1][:, s - t0:e - t0])
                    if b2 < S:
                        nc.vector.memset(qkv_sb[:, slot, b2:S], 0.0)
                # rope on q,k
                for slot in range(4):
                    for t0, t1 in _chunks(0, S, 512):
                        rp = mps.tile([128, 512], F32, name="rps", tag="qps0")
                        nc.tensor.matmul(rp[:], rot_sb[:],
                                         qkv_sb[:, slot, t0:t1],
                                         start=True, stop=True)
                        c1 = scrp.tile([128, 512], F32, name="ropec", tag="ropec")
                        nc.vector.tensor_mul(c1[:], qkv_sb[:, slot, t0:t1],
                                             cos_sb[:, t0:t1])
                        s1 = scrp.tile([128, 512], F32, name="ropes", tag="ropes")
                        nc.vector.tensor_mul(s1[:], rp[:], sin_sb[:, t0:t1])
                        nc.vector.tensor_add(qkv_sb[:, slot, t0:t1],
                                             c1[:], s1[:])
                # v -> token-major via PE transpose
                for hh in range(2):
                    for tt in range(16):
                        tp = mps.tile([128, 512], BF16, name="tps", tag="qps0")
                        nc.tensor.transpose(
                            tp[:, :128],
                            qkv_sb[:, 4 + hh, tt * 128:tt * 128 + 128],
                            ident[:])
                        nc.vector.tensor_copy(v_sb[:, tt, hh * 128:hh * 128 + 128],
                                              tp[:, :128])

        # ===== phase KV: E-sharded cross K/V projection + AllGather =====
        with ExitStack() as pK:
            kp_ = pK.enter_context(tc.tile_pool(name="kvp", bufs=1))
            kps = pK.enter_context(tc.tile_pool(name="kvps", bufs=2,
                                                space="PSUM"))
            enc_sb = kp_.tile([128, 8, 512], BF16)
            nc.sync.dma_start(enc_sb[:], r128(encTs.ap()))
            wk_sb = kp_.tile([128, 8, 512], BF16)
            nc.sync.dma_start(wk_sb[:], r128(wkh.ap()))
            wv_sb = kp_.tile([128, 8, 512], BF16)
            nc.sync.dma_start(wv_sb[:], r128(wvh.ap()))
            kcon_sb = kp_.tile([128, 4, 512], BF16)
            vcon_sb = kp_.tile([128, 4, 520], BF16)
            for db in range(4):
                ps = kps.tile([128, 512], F32, name="kcp", tag="kcp")
                for kc in range(8):
                    nc.tensor.matmul(ps[:], wk_sb[:, kc, db * 128:db * 128 + 128],
                                     enc_sb[:, kc, :],
                                     start=(kc == 0), stop=(kc == 7))
                nc.vector.tensor_copy(kcon_sb[:, db, :], ps[:])
            for hl in range(8):
                nc.vector.memset(vcon_sb[:, :, 65 * hl + 64], 1.0)
            for eb in range(4):
                ps = kps.tile([128, 512], F32, name="vcp", tag="kcp")
                for kc in range(8):
                    nc.tensor.matmul(ps[:], enc_sb[:, kc, eb * 128:eb * 128 + 128],
                                     wv_sb[:, kc, :],
                                     start=(kc == 0), stop=(kc == 7))
                for hl in range(8):
                    nc.vector.tensor_copy(
                        vcon_sb[:, eb, 65 * hl:65 * hl + 64],
                        ps[:, 64 * hl:64 * hl + 64])
            nc.sync.dma_start(r128(kvcon[0:512, 0:512]), kcon_sb[:])
            nc.sync.dma_start(r128(kvcon[512:1024, :]), vcon_sb[:])
        nc.gpsimd.collective_compute(
            "AllGather", mybir.AluOpType.bypass,
            replica_groups=[[0, 1, 2, 3], [4, 5, 6, 7]],
            ins=[kvcon.opt()], outs=[kvagg.opt()])

        # ===== phase B: self-attention (block-skip causal) =====
        with ExitStack() as pB:
            ap_ = pB.enter_context(tc.tile_pool(name="ap", bufs=3))
            aps = pB.enter_context(tc.tile_pool(name="aps", bufs=2, space="PSUM"))
            accp = pB.enter_context(tc.tile_pool(name="accp", bufs=1, space="PSUM"))
            for qc in range(4):
                t0 = qc * 512
                live = [kt for kt in range(16) if (qc, kt) in blocks]
                pss_ = [accp.tile([128, 512], F32, name=f"pssum{h}", tag=f"pssum{h}")
                        for h in range(2)]
                psc_ = [accp.tile([128, 512], F32, name=f"psctx{h}", tag=f"psctx{h}")
                        for h in range(2)]
                for kt in live:
                    cls, mix = blocks[(qc, kt)]
                    mt_ = None
                    if cls == 'M':
                        mt_ = ap_.tile([128, 512], BF16, name="mt", tag="mt")
                        nc.sync.dma_start(
                            mt_[:], maskmix.ap()[mix * 128:mix * 128 + 128, :])
                    for hh in range(2):
                        sc = aps.tile([128, 512], F32, name="sc", tag="sc")
                        nc.tensor.matmul(
                            sc[:], qkv_sb[:, 2 + hh, kt * 128:kt * 128 + 128],
                            qkv_sb[:, hh, t0:t0 + 512], start=True, stop=True)
                        if mt_ is not None:
                            nc.vector.tensor_add(sc[:], sc[:], mt_[:])
                        pr = ap_.tile([128, 512], BF16, name="pr", tag="pr")
                        nc.scalar.activation(pr[:], sc[:], EXP, scale=SC)
                        nc.tensor.matmul(pss_[hh][:], ones_bf[:], pr[:],
                                         start=(kt == live[0]),
                                         stop=(kt == live[-1]))
                        nc.tensor.matmul(
                            psc_[hh][:], v_sb[:, kt, hh * 128:hh * 128 + 128],
                            pr[:], start=(kt == live[0]), stop=(kt == live[-1]))
                for hh in range(2):
                    rc = ap_.tile([128, 512], F32, name="rc", tag="rc")
                    nc.vector.reciprocal(rc[:], pss_[hh][:])
                    nc.vector.tensor_mul(ctx_sb[:, hh, t0:t0 + 512],
                                         psc_[hh][:], rc[:])

        # ===== phase C: dense (routed, bf16) -> feature-split RS =====
        with ExitStack() as pC:
            dwp = pC.enter_context(tc.tile_pool(name="dwp", bufs=1))
            dps = pC.enter_context(tc.tile_pool(name="dps", bufs=2, space="PSUM"))
            dop = pC.enter_context(tc.tile_pool(name="dop", bufs=4))
            dwts = []
            for ex, wsrc in ((0, wd0), (1, wd1)):
                dwt = dwp.tile([128, 2, H], BF16, name=f"dw{ex}", tag=f"dw{ex}")
                nc.sync.dma_start(dwt[:], r128(wsrc.ap()))
                dwts.append(dwt)
            # dram view [p, half, slot*mt, n]: row = half*4096 + c*128 + p
            rb = lambda t: t[:].rearrange("(h c p) n -> p h c n", h=2, p=128)
            bnc_a = rb(bounce_a)
            bnc_b = rb(bounce_b)
            for half, bnc in ((0, bnc_a), (1, bnc_b)):
                for mt in range(8 * half, 8 * half + 8):
                    for ci in range(4):
                        t0 = ci * 512
                        sg = _segs(t0, t0 + 512, b0, b1, b2)
                        live = [x for x in sg if x[2]]
                        ot = dop.tile([128, 512], BF16, name="dot", tag="dot")
                        if live:
                            need = sorted({x for _, _, ex in live for x in ex})
                            pss_ = {}
                            for x in need:
                                ps = dps.tile([128, 512], F32, name=f"dpst{x}",
                                              tag=f"dpst{x}")
                                for kc in range(2):
                                    nc.tensor.matmul(
                                        ps[:],
                                        dwts[x][:, kc, mt * 128:mt * 128 + 128],
                                        ctx_sb[:, kc, t0:t0 + 512],
                                        start=(kc == 0), stop=(kc == 1))
                                pss_[x] = ps
                            for s, e, ex in sg:
                                if len(ex) == 2:
                                    nc.vector.tensor_add(ot[:, s - t0:e - t0],
                                                         pss_[0][:, s - t0:e - t0],
                                                         pss_[1][:, s - t0:e - t0])
                                elif ex:
                                    nc.vector.tensor_copy(
                                        ot[:, s - t0:e - t0],
                                        pss_[ex[0]][:, s - t0:e - t0])
                                else:
                                    nc.vector.memset(ot[:, s - t0:e - t0], 0.0)
                        else:
                            nc.vector.memset(ot[:], 0.0)
                        c_ = ci * 8 + (mt - 8 * half)
                        nc.sync.dma_start(bnc[:, :, c_:c_ + 1, :], ot[:])
                if half == 0:
                    nc.gpsimd.collective_compute(
                        "ReduceScatter", mybir.AluOpType.add,
                        replica_groups=[list(range(NC_))],
                        ins=[bounce_a.opt()], outs=[rs_a.opt()])
                else:
                    nc.gpsimd.collective_compute(
                        "ReduceScatter", mybir.AluOpType.add,
                        replica_groups=[list(range(NC_))],
                        ins=[bounce_b.opt()], outs=[rs_b.opt()])
        pABC.close()

        # ===== phase D1: h1 = rs + resid, rmsnorm, pairwise h1n AG =====
        dp = top.enter_context(tc.tile_pool(name="dp", bufs=1))
        h1_sb = dp.tile([128, 16, 256], F32)
        with ExitStack() as pD1:
            d1 = pD1.enter_context(tc.tile_pool(name="d1", bufs=1))
            d1ps = pD1.enter_context(tc.tile_pool(name="d1ps", bufs=1,
                                                  space="PSUM"))
            re_sb = d1.tile([128, 16, 256], F32)
            nc.sync.dma_start(re_sb[:], r128(resid.ap()))
            rsl_sb = d1.tile([128, 16, 256], BF16)
            nc.sync.dma_start(rsl_sb[:, 0:8, :], r128(rs_a[:]))
            nc.sync.dma_start(rsl_sb[:, 8:16, :], r128(rs_b[:]))
            pss = d1ps.tile([128, 256], F32, name="psd", tag="psd")
            for kc in range(16):
                nc.vector.tensor_add(h1_sb[:, kc, :],
                                     rsl_sb[:, kc, :], re_sb[:, kc, :])
                sq = scrp.tile([128, 256], BF16, name="sqd", tag="sqd")
                nc.scalar.activation(sq[:], h1_sb[:, kc, :], SQ)
                nc.tensor.matmul(pss[:], ones_bf[:], sq[:],
                                 start=(kc == 0), stop=(kc == 15))
            rms1 = d1.tile([128, 256], F32)
            nc.scalar.activation(rms1[:], pss[:], SQRT,
                                 scale=1.0 / H, bias=eps_sb[:])
            rinv = d1.tile([128, 256], F32)
            nc.vector.reciprocal(rinv[:], rms1[:])
            h1n_sb = d1.tile([128, 16, 256], BF16)
            for kc in range(16):
                nc.vector.tensor_mul(h1n_sb[:, kc, :],
                                     h1_sb[:, kc, :], rinv[:])
            nc.sync.dma_start(r128(h1n_bnc[:]), h1n_sb[:])
        nc.gpsimd.collective_compute(
            "AllGather", mybir.AluOpType.bypass,
            replica_groups=[[0, 4], [1, 5], [2, 6], [3, 7]],
            ins=[h1n_bnc.opt()], outs=[h1n_pair.opt()])

        # ===== phase D2: cross-attn, 8 heads x 512 tokens =====
        cctxp = top.enter_context(tc.tile_pool(name="cctxp", bufs=1))
        cctx_sb = cctxp.tile([128, 4, 512], BF16)
        with ExitStack() as pD2:
            d2 = pD2.enter_context(tc.tile_pool(name="d2", bufs=1))
            h1n5 = d2.tile([128, 16, 512], BF16)
            for r in range(2):
                nc.sync.dma_start(h1n5[:, :, 256 * r:256 * r + 256],
                                  r128(h1n_pair[H * r:H * (r + 1), :]))
            k_sb = d2.tile([128, 4, 2048], BF16)
            for hl in range(8):
                for src in range(4):
                    nc.sync.dma_start(
                        k_sb[64 * (hl % 2):64 * (hl % 2) + 64, hl // 2,
                             512 * src:512 * src + 512],
                        kvagg[1024 * src + 64 * hl:
                              1024 * src + 64 * hl + 64, 0:512])
            v_sb2 = d2.tile([128, 16, 520], BF16)
            for src in range(4):
                nc.sync.dma_start(
                    v_sb2[:, 4 * src:4 * src + 4, :],
                    r128(kvagg[1024 * src + 512:1024 * src + 1024, :]))
            wcq_sb = d2.tile([128, 16, 512], BF16)
            nc.sync.dma_start(wcq_sb[:], r128(wcqh.ap()))
            cq_sb = d2.tile([128, 4, 512], BF16)
            d2ps = pD2.enter_context(tc.tile_pool(name="d2ps", bufs=2,
                                                  space="PSUM"))
            for sl in range(4):
                ps = d2ps.tile([128, 512], F32, name="cqp", tag="cqp")
                for kc in range(16):
                    nc.tensor.matmul(ps[:],
                                     wcq_sb[:, kc, sl * 128:sl * 128 + 128],
                                     h1n5[:, kc, :],
                                     start=(kc == 0), stop=(kc == 15))
                nc.vector.tensor_copy(cq_sb[:, sl, :], ps[:])
            with ExitStack() as pD3:
                cap = pD3.enter_context(tc.tile_pool(name="cap", bufs=3))
                caps = pD3.enter_context(tc.tile_pool(name="caps", bufs=2,
                                                      space="PSUM"))
                cacc = pD3.enter_context(tc.tile_pool(name="cacc", bufs=1,
                                                      space="PSUM"))
                for wave in range(2):
                    psc = [cacc.tile([65, 512], F32, name=f"cps{j}", tag=f"cps{j}")
                           for j in range(4)]
                    for kt in range(16):
                        for j in range(4):
                            hl = 4 * wave + j
                            sc = caps.tile([128, 512], F32, name="csc", tag="csc")
                            nc.tensor.matmul(
                                sc[:],
                                k_sb[64 * (hl % 2):64 * (hl % 2) + 64, hl // 2,
                                     kt * 128:kt * 128 + 128],
                                cq_sb[64 * (hl % 2):64 * (hl % 2) + 64,
                                      hl // 2, :],
                                start=True, stop=True)
                            pr = cap.tile([128, 512], BF16, name="cpr", tag="cpr")
                            nc.scalar.activation(pr[:], sc[:], EXP, scale=CSC)
                            nc.tensor.matmul(
                                psc[j][:],
                                v_sb2[:, kt, 65 * hl:65 * hl + 65],
                                pr[:], start=(kt == 0), stop=(kt == 15))
                    for j in range(4):
                        hl = 4 * wave + j
                        dnf = cap.tile([1, 512], F32, name="dnf", tag="dnf")
                        nc.vector.tensor_copy(dnf[:], psc[j][64:65, :])
                        rcf = cap.tile([1, 512], F32, name="rcf", tag="rcf")
                        nc.vector.reciprocal(rcf[:], dnf[:])
                        rcb = cap.tile([1, 512], BF16, name="rcb", tag="rcb")
                        nc.vector.tensor_copy(rcb[:], rcf[:])
                        dnb = caps.tile([64, 512], F32, name="dnb", tag="csc")
                        nc.tensor.matmul(dnb[:], ones_bf[0:1, 0:64], rcb[:],
                                         start=True, stop=True)
                        rbf = cap.tile([64, 512], F32, name="rbf", tag="rbf")
                        nc.vector.tensor_copy(rbf[:], dnb[:])
                        po = 64 * (hl % 2)
                        nc.vector.tensor_mul(cctx_sb[po:po + 64, hl // 2, :],
                                             psc[j][0:64, :], rbf[:])

        # ===== phase D2b: cdense half-contraction -> pairwise RS =====
        with ExitStack() as pD4:
            d4 = pD4.enter_context(tc.tile_pool(name="d4", bufs=1))
            d4o = pD4.enter_context(tc.tile_pool(name="d4o", bufs=4))
            wcd_sb = d4.tile([128, 4, H], BF16)
            nc.sync.dma_start(wcd_sb[:], r128(wcdh.ap()))
            d4ps = pD4.enter_context(tc.tile_pool(name="d4ps", bufs=2,
                                                  space="PSUM"))
            cdp_t = cdpart[:].rearrange("(h c p) n -> p h c n", h=2, p=128)
            for mt in range(16):
                ps = d4ps.tile([128, 512], F32, name="cdp", tag="cdp")
                for kc in range(4):
                    nc.tensor.matmul(ps[:],
                                     wcd_sb[:, kc, mt * 128:mt * 128 + 128],
                                     cctx_sb[:, kc, :],
                                     start=(kc == 0), stop=(kc == 3))
                ot = d4o.tile([128, 512], BF16, name="cdo", tag="cdo")
                nc.vector.tensor_copy(ot[:], ps[:])
                nc.sync.dma_start(cdp_t[:, :, mt:mt + 1, :], ot[:])
        nc.gpsimd.collective_compute(
            "ReduceScatter", mybir.AluOpType.add,
            replica_groups=[[0, 4], [1, 5], [2, 6], [3, 7]],
            ins=[cdpart.opt()], outs=[cdsum.opt()])

        # ===== phase D3: h2 = h1 + cdsum, rmsnorm, split AG =====
        with ExitStack() as pD5:
            d5 = pD5.enter_context(tc.tile_pool(name="d5", bufs=1))
            cds_sb = d5.tile([128, 16, 256], BF16)
            nc.sync.dma_start(cds_sb[:], r128(cdsum[:]))
            h2_sb = d5.tile([128, 16, 256], F32)
            d5ps = pD5.enter_context(tc.tile_pool(name="d5ps", bufs=1,
                                                  space="PSUM"))
            pss2 = d5ps.tile([128, 256], F32, name="psd2", tag="psd2")
            for kc in range(16):
                nc.vector.tensor_add(h2_sb[:, kc, :], cds_sb[:, kc, :],
                                     h1_sb[:, kc, :])
                sq = scrp.tile([128, 256], BF16, name="sqd2", tag="sqd")
                nc.scalar.activation(sq[:], h2_sb[:, kc, :], SQ)
                nc.tensor.matmul(pss2[:], ones_bf[:], sq[:],
                                 start=(kc == 0), stop=(kc == 15))
            nc.sync.dma_start(r128(h2out.ap()), h2_sb[:])
            rms2 = d5.tile([128, 256], F32)
            nc.scalar.activation(rms2[:], pss2[:], SQRT,
                                 scale=1.0 / H, bias=eps_sb[:])
            rinv2 = d5.tile([128, 256], F32)
            nc.vector.reciprocal(rinv2[:], rms2[:])
            h2n_sb = d5.tile([128, 16, 256], BF16)
            for kc in range(16):
                nc.vector.tensor_mul(h2n_sb[:, kc, :],
                                     h2_sb[:, kc, :], rinv2[:])
            nc.sync.dma_start(r128(h2n_bnc_a[:]), h2n_sb[:, 0:8, :])
            nc.sync.dma_start(r128(h2n_bnc_b[:]), h2n_sb[:, 8:16, :])
        nc.gpsimd.collective_compute(
            "AllGather", mybir.AluOpType.bypass,
            replica_groups=[list(range(NC_))],
            ins=[h2n_bnc_a.opt()], outs=[h2na.opt()])
        nc.gpsimd.collective_compute(
            "AllGather", mybir.AluOpType.bypass,
            replica_groups=[list(range(NC_))],
            ins=[h2n_bnc_b.opt()], outs=[h2nb.opt()])

        # ===== phase F: MLP (routed by expert ranges, bf16) =====
        with ExitStack() as pF:
            fp = pF.enter_context(tc.tile_pool(name="fp", bufs=1))
            hn_sb = fp.tile([128, 16, S], BF16)
            for r in range(NC_):
                tr = 512 * (r % 4) + 256 * (r // 4)
                nc.sync.dma_start(hn_sb[:, 0:8, tr:tr + 256],
                                  r128(h2na[r * 1024:(r + 1) * 1024, :]))
                nc.sync.dma_start(hn_sb[:, 8:16, tr:tr + 256],
                                  r128(h2nb[r * 1024:(r + 1) * 1024, :]))
            fw = pF.enter_context(tc.tile_pool(name="fw", bufs=1))
            fps = pF.enter_context(tc.tile_pool(name="fps", bufs=1, space="PSUM"))
            fpd = pF.enter_context(tc.tile_pool(name="fpd", bufs=2, space="PSUM"))
            fac = pF.enter_context(tc.tile_pool(name="fac", bufs=2))
            fout = pF.enter_context(tc.tile_pool(name="fout", bufs=4))
            for ex, (lo, hi) in ((0, (0, b1)), (1, (b1, S))):
                gsrc = (wgu0, wgu1)[ex]
                dsrc = (wdn0, wdn1)[ex]
                dn_t = fw.tile([128, 6, H], BF16, name=f"dn{ex}", tag="dn")
                nc.sync.dma_start(dn_t[:], r128(dsrc.ap()))
                gwts = []
                for pi in range(6):
                    gw = 128 if pi < 5 else 48
                    gwt = fw.tile([128, 16, 256], BF16,
                                  name=f"guw{ex}{pi}", tag=f"guw{pi}")
                    nc.sync.dma_start(
                        gwt[:, :, :2 * gw],
                        r128(gsrc.ap()[:, pi * 256:pi * 256 + 2 * gw]))
                    gwts.append(gwt)
                for a0 in range(0, S, 512):
                    c0, c1 = max(a0, lo), min(a0 + 512, hi)
                    if c0 >= c1:
                        continue
                    t0_, W = a0, 512
                    eo, ew = c0 - a0, c1 - c0
                    act = fac.tile([128, 6, 512], BF16, name="act", tag="act")
                    for pi in range(6):
                        gw = 128 if pi < 5 else 48
                        gwt = gwts[pi]
                        pg = fps.tile([128, 512], F32, name="pg", tag="pg")
                        pu = fps.tile([128, 512], F32, name="pu", tag="pu")
                        for kc in range(16):
                            nc.tensor.matmul(pg[:gw, :W], gwt[:, kc, :gw],
                                             hn_sb[:, kc, t0_:t0_ + 512],
                                             start=(kc == 0), stop=(kc == 15))
                            nc.tensor.matmul(pu[:gw, :W], gwt[:, kc, gw:2 * gw],
                                             hn_sb[:, kc, t0_:t0_ + 512],
                                             start=(kc == 0), stop=(kc == 15))
                        gs = scrp.tile([128, 512], F32, name="gs", tag="gs")
                        nc.scalar.activation(gs[:gw, :W], pg[:gw, :W], SILU)
                        nc.vector.tensor_mul(act[:gw, pi, :W],
                                             gs[:gw, :W], pu[:gw, :W])
                    for mt in range(16):
                        pd = fpd.tile([128, 512], F32, name="pd", tag="pd")
                        for pi in range(6):
                            kw = 128 if pi < 5 else 48
                            nc.tensor.matmul(
                                pd[:, :W],
                                dn_t[:kw, pi, mt * 128:mt * 128 + 128],
                                act[:kw, pi, :W],
                                start=(pi == 0), stop=(pi == 5))
                        ot = fout.tile([128, 512], BF16, name="fot", tag="fot")
                        nc.vector.tensor_copy(ot[:, eo:eo + ew], pd[:, eo:eo + ew])
                        nc.sync.dma_start(
                            y.ap()[mt * 128:mt * 128 + 128, c0:c1],
                            ot[:, eo:eo + ew])
    nc.compile()
    return nc


_CACHE = {}


def kernel(**inputs):
    import ml_dtypes
    vm = np.asarray(inputs["vision_token_ids"]).astype(bool)
    lm = np.asarray(inputs["language_token_ids"]).astype(bool)
    g0 = np.where(vm & ~lm)[0]; g1 = np.where(vm & lm)[0]
    g2 = np.where(~vm & lm)[0]; g3 = np.where(~vm & ~lm)[0]
    perm = np.concatenate([g0, g1, g2, g3])
    b0 = len(g0); b1 = b0 + len(g1); b2 = b1 + len(g2)

    f32 = lambda x: np.ascontiguousarray(np.asarray(x, np.float32))
    bf = lambda x: np.ascontiguousarray(np.asarray(x).astype(ml_dtypes.bfloat16))
    pos = np.asarray(inputs["positions"]).astype(np.float32)
    half = HD // 2
    inv_freq = 1.0 / (ROPE_BASE ** (np.arange(half, dtype=np.float32) / half))
    fr = pos[:, None] * inv_freq[None, :]
    cos2 = np.concatenate([np.cos(fr)] * 2, 1).T[:, perm]
    sin2 = np.concatenate([np.sin(fr)] * 2, 1).T[:, perm]
    rot = np.zeros((HD, HD), np.float32)
    rot[np.arange(half), np.arange(half) + half] = -1.0
    rot[np.arange(half) + half, np.arange(half)] = 1.0
    op = np.asarray(inputs["positions"])[perm].astype(np.int64)

    # block classification: rows=keys [128kt,+128), cols=queries [512qc,+512)
    blocks = {}
    mix_tiles = []
    for qc in range(4):
        opq = op[512 * qc:512 * qc + 512]
        for kt in range(16):
            opk = op[128 * kt:128 * kt + 128]
            if opq.min() >= opk.max():
                blocks[(qc, kt)] = ('F', -1)
            elif opq.max() < opk.min():
                pass  # skip
            else:
                blocks[(qc, kt)] = ('M', len(mix_tiles))
                mix_tiles.append(
                    np.where(opq[None, :] >= opk[:, None], 0.0, -30000.0))
    maskmix = (np.concatenate(mix_tiles, 0) if mix_tiles
               else np.zeros((128, 512), np.float32))

    wln_in = f32(inputs["w_ln_in"])[:, None]
    wln_pa = f32(inputs["w_ln_post_attn"])[:, None]
    wln_pc = f32(inputs["w_ln_post_cross"])[:, None]
    wqkv = [f32(inputs["w_vis_qkv"]) * wln_in, f32(inputs["w_lang_qkv"]) * wln_in]
    wd = [f32(inputs["w_vis_dense"]), f32(inputs["w_lang_dense"])]
    wgu = [f32(inputs["w_vis_gate_up"]) * wln_pc,
           f32(inputs["w_lang_gate_up"]) * wln_pc]
    wdn = [f32(inputs["w_vis_down"]), f32(inputs["w_lang_down"])]
    wkvf = f32(inputs["w_cross_kv"])
    wcqf = f32(inputs["w_cross_q"]) * wln_pa
    wcdf = f32(inputs["w_cross_dense"])
    encTf = f32(inputs["encoder_embeds"]).T
    hTp = f32(inputs["hidden_states"]).T[:, perm].copy()

    def interleave(w):  # w [H, 2*IS] = [gate | up]
        cols = []
        for i in range(5):
            cols.append(w[:, 128 * i:128 * i + 128])
            cols.append(w[:, IS + 128 * i:IS + 128 * i + 128])
        cols.append(w[:, 640:IS]); cols.append(w[:, IS + 640:2 * IS])
        return np.ascontiguousarray(np.concatenate(cols, 1))

    bsig = tuple(sorted((k, v[0]) for k, v in blocks.items()))
    key = (b0, b1, b2, bsig)
    if key not in _CACHE:
        _CACHE.clear()
        _CACHE[key] = build_kernel(b0, b1, b2, blocks)
    nc = _CACHE[key]

    in_maps = []
    tslices = []
    for c in range(NC_):
        hh, g = c // 4, c % 4
        ts = slice(512 * g + 256 * hh, 512 * g + 256 * hh + 256)
        tslices.append(ts)
        qs = slice(256 * c, 256 * c + 256)
        m = dict(
            hT=bf(hTp),
            wqkv0=bf(np.concatenate([wqkv[0][:, qs], wqkv[0][:, H:][:, qs],
                                     wqkv[0][:, 2 * H:][:, qs]], 1)),
            wqkv1=bf(np.concatenate([wqkv[1][:, qs], wqkv[1][:, H:][:, qs],
                                     wqkv[1][:, 2 * H:][:, qs]], 1)),
            wd0=bf(wd[0][qs]), wd1=bf(wd[1][qs]),
            cos2=bf(cos2), sin2=bf(sin2), rotT=bf(rot.T),
            onesb=np.ones((128, 128), ml_dtypes.bfloat16),
            maskmix=bf(maskmix),
            resid=hTp[:, ts].copy(),
            encTs=bf(encTf[:, 512 * g:512 * g + 512]),
            wkh=bf(wkvf[:, :CC][:, 512 * hh:512 * hh + 512]),
            wvh=bf(wkvf[:, CC:][:, 512 * hh:512 * hh + 512]),
            wcqh=bf(wcqf[:, 512 * hh:512 * hh + 512]),
            wcdh=bf(wcdf[512 * hh:512 * hh + 512, :]),
            wgu0=bf(interleave(np.concatenate(
                [wgu[0][:, IS * c:IS * c + IS],
                 wgu[0][:, I + IS * c:I + IS * c + IS]], 1))),
            wgu1=bf(interleave(np.concatenate(
                [wgu[1][:, IS * c:IS * c + IS],
                 wgu[1][:, I + IS * c:I + IS * c + IS]], 1))),
            wdn0=bf(np.concatenate([wdn[0][IS * c:IS * c + IS],
                                    np.zeros((ISP - IS, H), np.float32)], 0)),
            wdn1=bf(np.concatenate([wdn[1][IS * c:IS * c + IS],
                                    np.zeros((ISP - IS, H), np.float32)], 0)),
        )
        in_maps.append(m)

    trace = bool(int(os.environ.get("KTRACE", "0")))
    res = run_bass_kernel_spmd(nc, in_maps, core_ids=list(range(NC_)),
                               trace=trace,
                               tmpdir=os.environ.get("KTRACE_DIR") or None)
    kernel.last_exec_ns = res.exec_time_ns
    kernel.last_trace = res.instructions_and_trace
    kernel.last_results = res.results
    kernel.last_tslices = tslices
    tot = res.results[0]["y"].astype(np.float64)
    for c in range(1, NC_):
        tot += res.results[c]["y"]
    for c in range(NC_):
        tot[:, tslices[c]] += res.results[c]["h2out"]
    out = np.empty((S, H), np.float32)
    out[perm, :] = tot.T.astype(np.float32)
    return out


# revision 30
# speedup vs baseline: 1.3243x; 1.0360x over previous
"""Trainium2 Bass kernel for nn_CogAgentDecoderLayer (8-core SPMD).

Feature-major activations [feat, tok] in permuted token order
(vis-only | both | lang-only | neither). TP plan per core c
(hh = c//4 head-half, g = c%4 token-group):
  - self-attn: heads split 2/core, causal block-skip, masks only for
    mixed diagonal blocks; dense row-parallel bf16 -> feature-split
    ReduceScatter x2 (RS_a overlaps dense 2nd half).
  - RS token slot T_c = [512*g + 256*hh, +256).
  - cross-attn: KV proj E-sharded + 4-group AllGather (issued early,
    hides behind self-attn); per-core 8 heads x 512 tokens (N=512
    matmuls); softmax denom folded into AV via 65th ones-row; cdense
    half-contraction + pairwise ReduceScatter.
  - MLP: I sliced 688/core, routed by expert; h2n AllGather split by
    feature halves so AG_b hides behind first-half gate_up. y bf16
    partials summed on host.
All matmuls bf16 (fp32 matmul is 2x rows on PE).
"""
import os
import numpy as np
from contextlib import ExitStack
from concourse import bacc, tile, mybir
from concourse.bass_utils import run_bass_kernel_spmd

NC_ = 8
S, E, H, NH, HD = 2048, 2048, 2048, 16, 128
CH, CC, CHD = 1024, 1024, 64
I = 5504
IS = I // NC_          # 688
ISP = 768              # padded to 6*128
EPS = 1e-5
ROPE_BASE = 10000.0
F32 = mybir.dt.float32
BF16 = mybir.dt.bfloat16


def _segs(lo, hi, b0, b1, b2):
    pts = sorted({lo, hi, *[b for b in (b0, b1, b2) if lo < b < hi]})
    out = []
    for s, e in zip(pts, pts[1:]):
        ex = []
        if s < b1:
            ex.append(0)
        if b0 <= s < b2:
            ex.append(1)
        out.append((s, e, ex))
    return out


def _chunks(lo, hi, w):
    out = []
    while lo < hi:
        out.append((lo, min(lo + w, hi)))
        lo += w
    return out


def build_kernel(b0, b1, b2, blocks):
    # blocks: dict (qc, kt) -> ('F', -1) or ('M', mix_idx); absent = skip
    n_mix = max(1, sum(1 for v in blocks.values() if v[0] == 'M'))
    nc = bacc.Bacc("TRN2", target_bir_lowering=False, debug=False,
                   num_devices=NC_)
    din = lambda n, sh, dt: nc.dram_tensor(n, sh, dt, kind="ExternalInput")
    hT = din("hT", [H, S], BF16)
    wqkv0 = din("wqkv0", [H, 768], BF16)
    wqkv1 = din("wqkv1", [H, 768], BF16)
    wd0 = din("wd0", [256, H], BF16)
    wd1 = din("wd1", [256, H], BF16)
    cos2 = din("cos2", [128, S], BF16)
    sin2 = din("sin2", [128, S], BF16)
    rotT = din("rotT", [128, 128], BF16)
    onesb = din("onesb", [128, 128], BF16)
    maskmix = din("maskmix", [n_mix * 128, 512], BF16)
    resid = din("resid", [H, 256], F32)
    encTs = din("encTs", [CH, 512], BF16)
    wkh = din("wkh", [CH, 512], BF16)
    wvh = din("wvh", [CH, 512], BF16)
    wcqh = din("wcqh", [H, 512], BF16)
    wcdh = din("wcdh", [512, H], BF16)
    wgu0 = din("wgu0", [H, 2 * IS], BF16)
    wgu1 = din("wgu1", [H, 2 * IS], BF16)
    wdn0 = din("wdn0", [ISP, H], BF16)
    wdn1 = din("wdn1", [ISP, H], BF16)
    y = nc.dram_tensor("y", [H, S], BF16, kind="ExternalOutput")

    SC = 1.0 / float(np.sqrt(HD))
    CSC = 1.0 / float(np.sqrt(CHD))
    EXP = mybir.ActivationFunctionType.Exp
    SQ = mybir.ActivationFunctionType.Square
    SQRT = mybir.ActivationFunctionType.Sqrt
    SILU = mybir.ActivationFunctionType.Silu
    r128 = lambda ap: ap.rearrange("(c p) n -> p c n", p=128)

    with tile.TileContext(nc) as tc, ExitStack() as top:
        const = top.enter_context(tc.tile_pool(name="const", bufs=1))
        ones_bf = const.tile([128, 128], BF16)
        nc.sync.dma_start(ones_bf[:], onesb.ap()[:])
        rot_sb = const.tile([128, 128], BF16)
        nc.sync.dma_start(rot_sb[:], rotT.ap()[:])
        from concourse.masks import make_identity
        ident = const.tile([128, 128], BF16)
        make_identity(nc, ident[:])
        cos_sb = const.tile([128, S], BF16)
        nc.sync.dma_start(cos_sb[:], cos2.ap()[:])
        sin_sb = const.tile([128, S], BF16)
        nc.sync.dma_start(sin_sb[:], sin2.ap()[:])
        eps_sb = const.tile([128, 1], F32)
        nc.vector.memset(eps_sb[:], EPS)

        dram = top.enter_context(tc.tile_pool(name="dram", bufs=1, space="DRAM"))
        bounce_a = dram.tile([8 * 1024, 256], BF16)
        bounce_b = dram.tile([8 * 1024, 256], BF16)
        rs_a = dram.tile([1024, 256], BF16)
        rs_b = dram.tile([1024, 256], BF16)
        h1n_bnc = dram.tile([H, 256], BF16)
        h1n_pair = dram.tile([2 * H, 256], BF16)
        kvcon = dram.tile([1024, 520], BF16)
        kvagg = dram.tile([4096, 520], BF16)
        cdpart = dram.tile([2 * H, 256], BF16)
        cdsum = dram.tile([H, 256], BF16)
        h2n_bnc_a = dram.tile([1024, 256], BF16)
        h2n_bnc_b = dram.tile([1024, 256], BF16)
        h2na = dram.tile([8 * 1024, 256], BF16, addr_space="Shared")
        h2nb = dram.tile([8 * 1024, 256], BF16, addr_space="Shared")
        h2out = nc.dram_tensor("h2out", [H, 256], F32, kind="ExternalOutput")

        scrp = top.enter_context(tc.tile_pool(name="scr", bufs=2))

        pABC = top.enter_context(ExitStack())
        qkp = pABC.enter_context(tc.tile_pool(name="qkp", bufs=1))
        qkv_sb = qkp.tile([128, 6, S], BF16)      # q0 q1 k0 k1 v0 v1
        v_sb = qkp.tile([128, 16, 256], BF16)     # token-major v
        ctxp = pABC.enter_context(tc.tile_pool(name="ctxp", bufs=1))
        ctx_sb = ctxp.tile([128, 2, S], BF16)
        hp = pABC.enter_context(ExitStack())
        hpool = hp.enter_context(tc.tile_pool(name="hp", bufs=1))
        h_sb = [hpool.tile([128, 16, 512], BF16, name=f"h{i}")
                for i in range(4)]
        for ci in range(4):
            nc.sync.dma_start(h_sb[ci][:],
                              r128(hT.ap()[:, ci * 512:ci * 512 + 512]))

        # ===== phase A: rmsnorm + QKV + rope + vT =====
        with ExitStack() as pA:
            with ExitStack() as pA1:
                nrm = pA1.enter_context(tc.tile_pool(name="nrm", bufs=2))
                nps = pA1.enter_context(tc.tile_pool(name="nps", bufs=2,
                                                     space="PSUM"))
                for ci, (t0, t1) in enumerate(_chunks(0, S, 512)):
                    pss = nps.tile([128, 512], F32, name="pss", tag="pss")
                    for kc in range(16):
                        sq = nrm.tile([128, 512], BF16, name="sq", tag="sq")
                        nc.scalar.activation(sq[:], h_sb[ci][:, kc, :], SQ)
                        nc.tensor.matmul(pss[:], ones_bf[:], sq[:],
                                         start=(kc == 0), stop=(kc == 15))
                    rms = nrm.tile([128, 512], F32, name="rms", tag="rms")
                    nc.scalar.activation(rms[:], pss[:], SQRT,
                                         scale=1.0 / H, bias=eps_sb[:])
                    rinv = nrm.tile([128, 512], F32, name="rinv", tag="rinv")
                    nc.vector.reciprocal(rinv[:], rms[:])
                    for kc in range(16):
                        nc.vector.tensor_mul(h_sb[ci][:, kc, :],
                                             h_sb[ci][:, kc, :], rinv[:])
            with ExitStack() as pA2:
                wp = pA2.enter_context(tc.tile_pool(name="wp", bufs=3))
                mps = pA2.enter_context(tc.tile_pool(name="mps", bufs=2,
                                                     space="PSUM"))
                for slot in range(6):
                    wts = []
                    for ex, wsrc in ((0, wqkv0), (1, wqkv1)):
                        wt = wp.tile([128, 16, 128], BF16,
                                     name=f"wq{ex}{slot}", tag=f"wq{ex}")
                        nc.sync.dma_start(
                            wt[:], r128(wsrc.ap()[:, slot * 128:slot * 128 + 128]))
                        wts.append(wt)
                    for ci, (t0, t1) in enumerate(_chunks(0, S, 512)):
                        sg = [x for x in _segs(t0, t1, b0, b1, b2) if x[2]]
                        if not sg:
                            continue
                        need = sorted({x for _, _, ex in sg for x in ex})
                        pss_ = {}
                        for x in need:
                            ps = mps.tile([128, 512], F32, name=f"qps{x}",
                                          tag=f"qps{x}")
                            for kc in range(16):
                                nc.tensor.matmul(ps[:], wts[x][:, kc, :],
                                                 h_sb[ci][:, kc, :],
                                                 start=(kc == 0), stop=(kc == 15))
                            pss_[x] = ps
                        for s, e, ex in sg:
                            if len(ex) == 1:
                                nc.vector.tensor_copy(qkv_sb[:, slot, s:e],
                                                      pss_[ex[0]][:, s - t0:e - t0])
                            else:
                                nc.vector.tensor_add(qkv_sb[:, slot, s:e],
                                                     pss_[0][:, s - t0:e - t0],
                                                     pss_[1][:, s - t0:e - t0])
                    if b2 < S:
                        nc.vector.memset(qkv_sb[:, slot, b2:S], 0.0)
                # rope on q,k
                for slot in range(4):
                    for t0, t1 in _chunks(0, S, 512):
                        rp = mps.tile([128, 512], F32, name="rps", tag="qps0")
                        nc.tensor.matmul(rp[:], rot_sb[:],
                                         qkv_sb[:, slot, t0:t1],
                                         start=True, stop=True)
                        c1 = scrp.tile([128, 512], F32, name="ropec", tag="ropec")
                        nc.vector.tensor_mul(c1[:], qkv_sb[:, slot, t0:t1],
                                             cos_sb[:, t0:t1])
                        s1 = scrp.tile([128, 512], F32, name="ropes", tag="ropes")
                        nc.vector.tensor_mul(s1[:], rp[:], sin_sb[:, t0:t1])
                        nc.vector.tensor_add(qkv_sb[:, slot, t0:t1],
                                             c1[:], s1[:])
                # v -> token-major via PE transpose
                for hh in range(2):
                    for tt in range(16):
                        tp = mps.tile([128, 512], BF16, name="tps", tag="qps0")
                        nc.tensor.transpose(
                            tp[:, :128],
                            qkv_sb[:, 4 + hh, tt * 128:tt * 128 + 128],
                            ident[:])
                        nc.vector.tensor_copy(v_sb[:, tt, hh * 128:hh * 128 + 128],
                                              tp[:, :128])

        hp.close()
        # ===== phase KV: E-sharded cross K/V projection + AllGather =====
        with ExitStack() as pK:
            kp_ = pK.enter_context(tc.tile_pool(name="kvp", bufs=1))
            kps = pK.enter_context(tc.tile_pool(name="kvps", bufs=2,
                                                space="PSUM"))
            enc_sb = kp_.tile([128, 8, 512], BF16)
            nc.sync.dma_start(enc_sb[:], r128(encTs.ap()))
            wk_sb = kp_.tile([128, 8, 512], BF16)
            nc.sync.dma_start(wk_sb[:], r128(wkh.ap()))
            wv_sb = kp_.tile([128, 8, 512], BF16)
            nc.sync.dma_start(wv_sb[:], r128(wvh.ap()))
            kcon_sb = kp_.tile([128, 4, 512], BF16)
            vcon_sb = kp_.tile([128, 4, 520], BF16)
            for db in range(4):
                ps = kps.tile([128, 512], F32, name="kcp", tag="kcp")
                for kc in range(8):
                    nc.tensor.matmul(ps[:], wk_sb[:, kc, db * 128:db * 128 + 128],
                                     enc_sb[:, kc, :],
                                     start=(kc == 0), stop=(kc == 7))
                nc.vector.tensor_copy(kcon_sb[:, db, :], ps[:])
            for hl in range(8):
                nc.vector.memset(vcon_sb[:, :, 65 * hl + 64], 1.0)
            for eb in range(4):
                ps = kps.tile([128, 512], F32, name="vcp", tag="kcp")
                for kc in range(8):
                    nc.tensor.matmul(ps[:], enc_sb[:, kc, eb * 128:eb * 128 + 128],
                                     wv_sb[:, kc, :],
                                     start=(kc == 0), stop=(kc == 7))
                for hl in range(8):
                    nc.vector.tensor_copy(
                        vcon_sb[:, eb, 65 * hl:65 * hl + 64],
                        ps[:, 64 * hl:64 * hl + 64])
            nc.sync.dma_start(r128(kvcon[0:512, 0:512]), kcon_sb[:])
            nc.sync.dma_start(r128(kvcon[512:1024, :]), vcon_sb[:])
        nc.gpsimd.collective_compute(
            "AllGather", mybir.AluOpType.bypass,
            replica_groups=[[0, 1, 2, 3], [4, 5, 6, 7]],
            ins=[kvcon.opt()], outs=[kvagg.opt()])

        # ===== phase B: self-attention (block-skip causal) =====
        with ExitStack() as pB:
            ap_ = pB.enter_context(tc.tile_pool(name="ap", bufs=3))
            aps = pB.enter_context(tc.tile_pool(name="aps", bufs=2, space="PSUM"))
            accp = pB.enter_context(tc.tile_pool(name="accp", bufs=2, space="PSUM"))
            mtiles = {}
            for qc in range(4):
                t0 = qc * 512
                live = [kt for kt in range(16) if (qc, kt) in blocks]
                for hh in range(2):
                    pss = accp.tile([128, 512], F32, name="pssum", tag="pssum")
                    psc = accp.tile([128, 512], F32, name="psctx", tag="psctx")
                    for kt in live:
                        cls, mix = blocks[(qc, kt)]
                        mt_ = None
                        if cls == 'M':
                            if hh == 0:
                                mt_ = ap_.tile([128, 512], BF16, name="mt",
                                               tag=f"mt{mix % 4}")
                                nc.sync.dma_start(
                                    mt_[:],
                                    maskmix.ap()[mix * 128:mix * 128 + 128, :])
                                mtiles[mix] = mt_
                            else:
                                mt_ = mtiles[mix]
                        sc = aps.tile([128, 512], F32, name="sc", tag="sc")
                        nc.tensor.matmul(
                            sc[:], qkv_sb[:, 2 + hh, kt * 128:kt * 128 + 128],
                            qkv_sb[:, hh, t0:t0 + 512], start=True, stop=True)
                        if mt_ is not None:
                            nc.vector.tensor_add(sc[:], sc[:], mt_[:])
                        pr = ap_.tile([128, 512], BF16, name="pr", tag="pr")
                        nc.scalar.activation(pr[:], sc[:], EXP, scale=SC)
                        nc.tensor.matmul(pss[:], ones_bf[:], pr[:],
                                         start=(kt == live[0]),
                                         stop=(kt == live[-1]))
                        nc.tensor.matmul(
                            psc[:], v_sb[:, kt, hh * 128:hh * 128 + 128],
                            pr[:], start=(kt == live[0]), stop=(kt == live[-1]))
                    rc = ap_.tile([128, 512], F32, name="rc", tag="rc")
                    nc.vector.reciprocal(rc[:], pss[:])
                    nc.vector.tensor_mul(ctx_sb[:, hh, t0:t0 + 512],
                                         psc[:], rc[:])

        # ===== phase C: dense (routed, bf16) -> feature-split RS =====
        with ExitStack() as pC:
            dwp = pC.enter_context(tc.tile_pool(name="dwp", bufs=1))
            dps = pC.enter_context(tc.tile_pool(name="dps", bufs=2, space="PSUM"))
            dop = pC.enter_context(tc.tile_pool(name="dop", bufs=4))
            dwts = []
            for ex, wsrc in ((0, wd0), (1, wd1)):
                dwt = dwp.tile([128, 2, H], BF16, name=f"dw{ex}", tag=f"dw{ex}")
                nc.sync.dma_start(dwt[:], r128(wsrc.ap()))
                dwts.append(dwt)
            # dram view [p, half, slot*mt, n]: row = half*4096 + c*128 + p
            rb = lambda t: t[:].rearrange("(h c p) n -> p h c n", h=2, p=128)
            bnc_a = rb(bounce_a)
            bnc_b = rb(bounce_b)
            for half, bnc in ((0, bnc_a), (1, bnc_b)):
                for mt in range(8 * half, 8 * half + 8):
                    for ci in range(4):
                        t0 = ci * 512
                        sg = _segs(t0, t0 + 512, b0, b1, b2)
                        live = [x for x in sg if x[2]]
                        ot = dop.tile([128, 512], BF16, name="dot", tag="dot")
                        if live:
                            need = sorted({x for _, _, ex in live for x in ex})
                            pss_ = {}
                            for x in need:
                                ps = dps.tile([128, 512], F32, name=f"dpst{x}",
                                              tag=f"dpst{x}")
                                for kc in range(2):
                                    nc.tensor.matmul(
                                        ps[:],
                                        dwts[x][:, kc, mt * 128:mt * 128 + 128],
                                        ctx_sb[:, kc, t0:t0 + 512],
                                        start=(kc == 0), stop=(kc == 1))
                                pss_[x] = ps
                            for s, e, ex in sg:
                                if len(ex) == 2:
                                    nc.vector.tensor_add(ot[:, s - t0:e - t0],
                                                         pss_[0][:, s - t0:e - t0],
                                                         pss_[1][:, s - t0:e - t0])
                                elif ex:
                                    nc.vector.tensor_copy(
                                        ot[:, s - t0:e - t0],
                                        pss_[ex[0]][:, s - t0:e - t0])
                                else:
                                    nc.vector.memset(ot[:, s - t0:e - t0], 0.0)
                        else:
                            nc.vector.memset(ot[:], 0.0)
                        c_ = ci * 8 + (mt - 8 * half)
                        nc.sync.dma_start(bnc[:, :, c_:c_ + 1, :], ot[:])
                if half == 0:
                    nc.gpsimd.collective_compute(
                        "ReduceScatter", mybir.AluOpType.add,
                        replica_groups=[list(range(NC_))],
                        ins=[bounce_a.opt()], outs=[rs_a.opt()])
                else:
                    nc.gpsimd.collective_compute(
                        "ReduceScatter", mybir.AluOpType.add,
                        replica_groups=[list(range(NC_))],
                        ins=[bounce_b.opt()], outs=[rs_b.opt()])
        pABC.close()

        # ===== phase D1: h1 = rs + resid, rmsnorm, pairwise h1n AG =====
        pDP = top.enter_context(ExitStack())
        dp = pDP.enter_context(tc.tile_pool(name="dp", bufs=1))
        h1_sb = dp.tile([128, 16, 256], F32)
        with ExitStack() as pD1:
            d1 = pD1.enter_context(tc.tile_pool(name="d1", bufs=1))
            d1ps = pD1.enter_context(tc.tile_pool(name="d1ps", bufs=1,
                                                  space="PSUM"))
            re_sb = d1.tile([128, 16, 256], F32)
            nc.sync.dma_start(re_sb[:], r128(resid.ap()))
            rsl_sb = d1.tile([128, 16, 256], BF16)
            nc.gpsimd.dma_start(rsl_sb[:, 0:8, :], r128(rs_a[:]))
            nc.gpsimd.dma_start(rsl_sb[:, 8:16, :], r128(rs_b[:]))
            pss = d1ps.tile([128, 256], F32, name="psd", tag="psd")
            for kc in range(16):
                nc.vector.tensor_add(h1_sb[:, kc, :],
                                     rsl_sb[:, kc, :], re_sb[:, kc, :])
                sq = scrp.tile([128, 256], BF16, name="sqd", tag="sqd")
                nc.scalar.activation(sq[:], h1_sb[:, kc, :], SQ)
                nc.tensor.matmul(pss[:], ones_bf[:], sq[:],
                                 start=(kc == 0), stop=(kc == 15))
            rms1 = d1.tile([128, 256], F32)
            nc.scalar.activation(rms1[:], pss[:], SQRT,
                                 scale=1.0 / H, bias=eps_sb[:])
            rinv = d1.tile([128, 256], F32)
            nc.vector.reciprocal(rinv[:], rms1[:])
            h1n_sb = d1.tile([128, 16, 256], BF16)
            for kc in range(16):
                nc.vector.tensor_mul(h1n_sb[:, kc, :],
                                     h1_sb[:, kc, :], rinv[:])
            nc.gpsimd.dma_start(r128(h1n_bnc[:]), h1n_sb[:])
        nc.gpsimd.collective_compute(
            "AllGather", mybir.AluOpType.bypass,
            replica_groups=[[0, 4], [1, 5], [2, 6], [3, 7]],
            ins=[h1n_bnc.opt()], outs=[h1n_pair.opt()])

        # ===== phase D2: cross-attn, 8 heads x 512 tokens =====
        pCC = top.enter_context(ExitStack())
        cctxp = pCC.enter_context(tc.tile_pool(name="cctxp", bufs=1))
        cctx_sb = cctxp.tile([128, 4, 512], BF16)
        with ExitStack() as pD2:
            d2 = pD2.enter_context(tc.tile_pool(name="d2", bufs=1))
            k_sb = d2.tile([128, 4, 2048], BF16)
            for hl in range(8):
                for src_ in range(4):
                    nc.sync.dma_start(
                        k_sb[64 * (hl % 2):64 * (hl % 2) + 64, hl // 2,
                             512 * src_:512 * src_ + 512],
                        kvagg[1024 * src_ + 64 * hl:
                              1024 * src_ + 64 * hl + 64, 0:512])
            v_sb2 = d2.tile([128, 16, 520], BF16)
            for src_ in range(4):
                nc.sync.dma_start(
                    v_sb2[:, 4 * src_:4 * src_ + 4, :],
                    r128(kvagg[1024 * src_ + 512:1024 * src_ + 1024, :]))
            wcq_sb = d2.tile([128, 16, 512], BF16)
            nc.sync.dma_start(wcq_sb[:], r128(wcqh.ap()))
            h1n5 = d2.tile([128, 16, 512], BF16)
            for r in range(2):
                nc.gpsimd.dma_start(h1n5[:, :, 256 * r:256 * r + 256],
                                    r128(h1n_pair[H * r:H * (r + 1), :]))
            cq_sb = d2.tile([128, 4, 512], BF16)
            d2ps = pD2.enter_context(tc.tile_pool(name="d2ps", bufs=2,
                                                  space="PSUM"))
            for sl in range(4):
                ps = d2ps.tile([128, 512], F32, name="cqp", tag="cqp")
                for kc in range(16):
                    nc.tensor.matmul(ps[:],
                                     wcq_sb[:, kc, sl * 128:sl * 128 + 128],
                                     h1n5[:, kc, :],
                                     start=(kc == 0), stop=(kc == 15))
                nc.vector.tensor_copy(cq_sb[:, sl, :], ps[:])
            with ExitStack() as pD3:
                cap = pD3.enter_context(tc.tile_pool(name="cap", bufs=3))
                caps = pD3.enter_context(tc.tile_pool(name="caps", bufs=2,
                                                      space="PSUM"))
                cacc = pD3.enter_context(tc.tile_pool(name="cacc", bufs=1,
                                                      space="PSUM"))
                for wave in range(2):
                    psc = [cacc.tile([65, 512], F32, name=f"cps{j}", tag=f"cps{j}")
                           for j in range(4)]
                    for kt in range(16):
                        for j in range(4):
                            hl = 4 * wave + j
                            sc = caps.tile([128, 512], F32, name="csc", tag="csc")
                            nc.tensor.matmul(
                                sc[:],
                                k_sb[64 * (hl % 2):64 * (hl % 2) + 64, hl // 2,
                                     kt * 128:kt * 128 + 128],
                                cq_sb[64 * (hl % 2):64 * (hl % 2) + 64,
                                      hl // 2, :],
                                start=True, stop=True)
                            pr = cap.tile([128, 512], BF16, name="cpr", tag="cpr")
                            nc.scalar.activation(pr[:], sc[:], EXP, scale=CSC)
                            nc.tensor.matmul(
                                psc[j][:],
                                v_sb2[:, kt, 65 * hl:65 * hl + 65],
                                pr[:], start=(kt == 0), stop=(kt == 15))
                    for j in range(4):
                        hl = 4 * wave + j
                        dnf = cap.tile([1, 512], F32, name="dnf", tag="dnf")
                        nc.vector.tensor_copy(dnf[:], psc[j][64:65, :])
                        rcf = cap.tile([1, 512], F32, name="rcf", tag="rcf")
                        nc.vector.reciprocal(rcf[:], dnf[:])
                        rcb = cap.tile([1, 512], BF16, name="rcb", tag="rcb")
                        nc.vector.tensor_copy(rcb[:], rcf[:])
                        dnb = caps.tile([64, 512], F32, name="dnb", tag="csc")
                        nc.tensor.matmul(dnb[:], ones_bf[0:1, 0:64], rcb[:],
                                         start=True, stop=True)
                        rbf = cap.tile([64, 512], F32, name="rbf", tag="rbf")
                        nc.vector.tensor_copy(rbf[:], dnb[:])
                        po = 64 * (hl % 2)
                        nc.vector.tensor_mul(cctx_sb[po:po + 64, hl // 2, :],
                                             psc[j][0:64, :], rbf[:])

        # ===== phase D2b: cdense half-contraction -> pairwise RS =====
        with ExitStack() as pD4:
            d4 = pD4.enter_context(tc.tile_pool(name="d4", bufs=1))
            d4o = pD4.enter_context(tc.tile_pool(name="d4o", bufs=4))
            wcd_sb = d4.tile([128, 4, H], BF16)
            nc.sync.dma_start(wcd_sb[:], r128(wcdh.ap()))
            d4ps = pD4.enter_context(tc.tile_pool(name="d4ps", bufs=2,
                                                  space="PSUM"))
            cdp_t = cdpart[:].rearrange("(h c p) n -> p h c n", h=2, p=128)
            for mt in range(16):
                ps = d4ps.tile([128, 512], F32, name="cdp", tag="cdp")
                for kc in range(4):
                    nc.tensor.matmul(ps[:],
                                     wcd_sb[:, kc, mt * 128:mt * 128 + 128],
                                     cctx_sb[:, kc, :],
                                     start=(kc == 0), stop=(kc == 3))
                ot = d4o.tile([128, 512], BF16, name="cdo", tag="cdo")
                nc.vector.tensor_copy(ot[:], ps[:])
                nc.sync.dma_start(cdp_t[:, :, mt:mt + 1, :], ot[:])
        pCC.close()
        nc.gpsimd.collective_compute(
            "ReduceScatter", mybir.AluOpType.add,
            replica_groups=[[0, 4], [1, 5], [2, 6], [3, 7]],
            ins=[cdpart.opt()], outs=[cdsum.opt()])

        # ===== phase D3: h2 = h1 + cdsum, rmsnorm, split AG =====
        with ExitStack() as pD5:
            d5 = pD5.enter_context(tc.tile_pool(name="d5", bufs=1))
            cds_sb = d5.tile([128, 16, 256], BF16)
            nc.gpsimd.dma_start(cds_sb[:], r128(cdsum[:]))
            h2_sb = d5.tile([128, 16, 256], F32)
            d5ps = pD5.enter_context(tc.tile_pool(name="d5ps", bufs=1,
                                                  space="PSUM"))
            pss2 = d5ps.tile([128, 256], F32, name="psd2", tag="psd2")
            for kc in range(16):
                nc.vector.tensor_add(h2_sb[:, kc, :], cds_sb[:, kc, :],
                                     h1_sb[:, kc, :])
                sq = scrp.tile([128, 256], BF16, name="sqd2", tag="sqd")
                nc.scalar.activation(sq[:], h2_sb[:, kc, :], SQ)
                nc.tensor.matmul(pss2[:], ones_bf[:], sq[:],
                                 start=(kc == 0), stop=(kc == 15))
            nc.gpsimd.dma_start(r128(h2out.ap()), h2_sb[:])
            rms2 = d5.tile([128, 256], F32)
            nc.scalar.activation(rms2[:], pss2[:], SQRT,
                                 scale=1.0 / H, bias=eps_sb[:])
            rinv2 = d5.tile([128, 256], F32)
            nc.vector.reciprocal(rinv2[:], rms2[:])
            h2n_sb = d5.tile([128, 16, 256], BF16)
            for kc in range(16):
                nc.vector.tensor_mul(h2n_sb[:, kc, :],
                                     h2_sb[:, kc, :], rinv2[:])
            nc.gpsimd.dma_start(r128(h2n_bnc_a[:]), h2n_sb[:, 0:8, :])
            nc.gpsimd.dma_start(r128(h2n_bnc_b[:]), h2n_sb[:, 8:16, :])
        pDP.close()
        # MLP expert-0 weight prefetch: no deps, so these flow on the sync
        # queue during pairRS/D3/AG. Expert-1 tiles reuse the same tags, so
        # their DMAs are issued inside phase F (a tag-reuse wait here would
        # stall the sync queue pre-AG).
        fw = top.enter_context(tc.tile_pool(name="fw", bufs=1))

        def load_mlp_w(ex, gsrc, dsrc):
            dn_t = fw.tile([128, 6, H], BF16, name=f"dn{ex}", tag="dn")
            nc.sync.dma_start(dn_t[:], r128(dsrc.ap()))
            gws = []
            for pi in range(6):
                gw = 128 if pi < 5 else 48
                gwt = fw.tile([128, 16, 256], BF16,
                              name=f"guw{ex}{pi}", tag=f"guw{pi}")
                nc.sync.dma_start(
                    gwt[:, :, :2 * gw],
                    r128(gsrc.ap()[:, pi * 256:pi * 256 + 2 * gw]))
                gws.append(gwt)
            return dn_t, gws

        dn_ts = {}
        gw_ts = {}
        dn_ts[0], gw_ts[0] = load_mlp_w(0, wgu0, wdn0)
        nc.gpsimd.collective_compute(
            "AllGather", mybir.AluOpType.bypass,
            replica_groups=[list(range(NC_))],
            ins=[h2n_bnc_a.opt()], outs=[h2na.opt()])
        nc.gpsimd.collective_compute(
            "AllGather", mybir.AluOpType.bypass,
            replica_groups=[list(range(NC_))],
            ins=[h2n_bnc_b.opt()], outs=[h2nb.opt()])

        # ===== phase F: MLP (routed by expert ranges, bf16) =====
        with ExitStack() as pF:
            fp = pF.enter_context(tc.tile_pool(name="fp", bufs=1))
            hn_sb = fp.tile([128, 16, S], BF16)
            for r in range(NC_):
                tr = 512 * (r % 4) + 256 * (r // 4)
                nc.gpsimd.dma_start(hn_sb[:, 0:8, tr:tr + 256],
                                    r128(h2na[r * 1024:(r + 1) * 1024, :]))
                nc.gpsimd.dma_start(hn_sb[:, 8:16, tr:tr + 256],
                                    r128(h2nb[r * 1024:(r + 1) * 1024, :]))
            fps = pF.enter_context(tc.tile_pool(name="fps", bufs=2, space="PSUM"))
            fpd = pF.enter_context(tc.tile_pool(name="fpd", bufs=2, space="PSUM"))
            fac = pF.enter_context(tc.tile_pool(name="fac", bufs=2))
            fout = pF.enter_context(tc.tile_pool(name="fout", bufs=4))
            for ex, (lo, hi) in ((0, (0, b1)), (1, (b1, S))):
                if ex not in dn_ts:
                    dn_ts[ex], gw_ts[ex] = load_mlp_w(ex, (wgu0, wgu1)[ex],
                                                      (wdn0, wdn1)[ex])
                dn_t = dn_ts[ex]
                gwts = gw_ts[ex]
                for a0 in range(0, S, 512):
                    c0, c1 = max(a0, lo), min(a0 + 512, hi)
                    if c0 >= c1:
                        continue
                    t0_, W = a0, 512
                    eo, ew = c0 - a0, c1 - c0
                    act = fac.tile([128, 6, 512], BF16, name="act", tag="act")
                    for pi in range(6):
                        gw = 128 if pi < 5 else 48
                        gwt = gwts[pi]
                        pg = fps.tile([128, 512], F32, name="pg", tag="pg")
                        pu = fps.tile([128, 512], F32, name="pu", tag="pu")
                        for kc in range(16):
                            nc.tensor.matmul(pg[:gw, :W], gwt[:, kc, :gw],
                                             hn_sb[:, kc, t0_:t0_ + 512],
                                             start=(kc == 0), stop=(kc == 15))
                            nc.tensor.matmul(pu[:gw, :W], gwt[:, kc, gw:2 * gw],
                                             hn_sb[:, kc, t0_:t0_ + 512],
                                             start=(kc == 0), stop=(kc == 15))
                        gs = scrp.tile([128, 512], F32, name="gs", tag="gs")
                        nc.scalar.activation(gs[:gw, :W], pg[:gw, :W], SILU)
                        nc.vector.tensor_mul(act[:gw, pi, :W],
                                             gs[:gw, :W], pu[:gw, :W])
                    for mt in range(16):
                        pd = fpd.tile([128, 512], F32, name="pd", tag="pd")
                        for pi in range(6):
                            kw = 128 if pi < 5 else 48
                            nc.tensor.matmul(
                                pd[:, :W],
                                dn_t[:kw, pi, mt * 128:mt * 128 + 128],
                                act[:kw, pi, :W],
                                start=(pi == 0), stop=(pi == 5))
                        ot = fout.tile([128, 512], BF16, name="fot", tag="fot")
                        nc.vector.tensor_copy(ot[:, eo:eo + ew], pd[:, eo:eo + ew])
                        nc.sync.dma_start(
                            y.ap()[mt * 128:mt * 128 + 128, c0:c1],
                            ot[:, eo:eo + ew])
    nc.compile()
    return nc


_CACHE = {}


def kernel(**inputs):
    import ml_dtypes
    vm = np.asarray(inputs["vision_token_ids"]).astype(bool)
    lm = np.asarray(inputs["language_token_ids"]).astype(bool)
    g0 = np.where(vm & ~lm)[0]; g1 = np.where(vm & lm)[0]
    g2 = np.where(~vm & lm)[0]; g3 = np.where(~vm & ~lm)[0]
    perm = np.concatenate([g0, g1, g2, g3])
    b0 = len(g0); b1 = b0 + len(g1); b2 = b1 + len(g2)

    f32 = lambda x: np.ascontiguousarray(np.asarray(x, np.float32))
    bf = lambda x: np.ascontiguousarray(np.asarray(x).astype(ml_dtypes.bfloat16))
    pos = np.asarray(inputs["positions"]).astype(np.float32)
    half = HD // 2
    inv_freq = 1.0 / (ROPE_BASE ** (np.arange(half, dtype=np.float32) / half))
    fr = pos[:, None] * inv_freq[None, :]
    cos2 = np.concatenate([np.cos(fr)] * 2, 1).T[:, perm]
    sin2 = np.concatenate([np.sin(fr)] * 2, 1).T[:, perm]
    rot = np.zeros((HD, HD), np.float32)
    rot[np.arange(half), np.arange(half) + half] = -1.0
    rot[np.arange(half) + half, np.arange(half)] = 1.0
    op = np.asarray(inputs["positions"])[perm].astype(np.int64)

    # block classification: rows=keys [128kt,+128), cols=queries [512qc,+512)
    blocks = {}
    mix_tiles = []
    for qc in range(4):
        opq = op[512 * qc:512 * qc + 512]
        for kt in range(16):
            opk = op[128 * kt:128 * kt + 128]
            if opq.min() >= opk.max():
                blocks[(qc, kt)] = ('F', -1)
            elif opq.max() < opk.min():
                pass  # skip
            else:
                blocks[(qc, kt)] = ('M', len(mix_tiles))
                mix_tiles.append(
                    np.where(opq[None, :] >= opk[:, None], 0.0, -30000.0))
    maskmix = (np.concatenate(mix_tiles, 0) if mix_tiles
               else np.zeros((128, 512), np.float32))

    wln_in = f32(inputs["w_ln_in"])[:, None]
    wln_pa = f32(inputs["w_ln_post_attn"])[:, None]
    wln_pc = f32(inputs["w_ln_post_cross"])[:, None]
    wqkv = [f32(inputs["w_vis_qkv"]) * wln_in, f32(inputs["w_lang_qkv"]) * wln_in]
    wd = [f32(inputs["w_vis_dense"]), f32(inputs["w_lang_dense"])]
    wgu = [f32(inputs["w_vis_gate_up"]) * wln_pc,
           f32(inputs["w_lang_gate_up"]) * wln_pc]
    wdn = [f32(inputs["w_vis_down"]), f32(inputs["w_lang_down"])]
    wkvf = f32(inputs["w_cross_kv"])
    wcqf = f32(inputs["w_cross_q"]) * wln_pa
    wcdf = f32(inputs["w_cross_dense"])
    encTf = f32(inputs["encoder_embeds"]).T
    hTp = f32(inputs["hidden_states"]).T[:, perm].copy()

    def interleave(w):  # w [H, 2*IS] = [gate | up]
        cols = []
        for i in range(5):
            cols.append(w[:, 128 * i:128 * i + 128])
            cols.append(w[:, IS + 128 * i:IS + 128 * i + 128])
        cols.append(w[:, 640:IS]); cols.append(w[:, IS + 640:2 * IS])
        return np.ascontiguousarray(np.concatenate(cols, 1))

    bsig = tuple(sorted((k, v[0]) for k, v in blocks.items()))
    key = (b0, b1, b2, bsig)
    if key not in _CACHE:
        _CACHE.clear()
        _CACHE[key] = build_kernel(b0, b1, b2, blocks)
    nc = _CACHE[key]

    in_maps = []
    tslices = []
    for c in range(NC_):
        hh, g = c // 4, c % 4
        ts = slice(512 * g + 256 * hh, 512 * g + 256 * hh + 256)
        tslices.append(ts)
        qs = slice(256 * c, 256 * c + 256)
        m = dict(
            hT=bf(hTp),
            wqkv0=bf(np.concatenate([wqkv[0][:, qs], wqkv[0][:, H:][:, qs],
                                     wqkv[0][:, 2 * H:][:, qs]], 1)),
            wqkv1=bf(np.concatenate([wqkv[1][:, qs], wqkv[1][:, H:][:, qs],
                                     wqkv[1][:, 2 * H:][:, qs]], 1)),
            wd0=bf(wd[0][qs]), wd1=bf(wd[1][qs]),
            cos2=bf(cos2), sin2=bf(sin2), rotT=bf(rot.T),
            onesb=np.ones((128, 128), ml_dtypes.bfloat16),
            maskmix=bf(maskmix),
            resid=hTp[:, ts].copy(),
            encTs=bf(encTf[:, 512 * g:512 * g + 512]),
            wkh=bf(wkvf[:, :CC][:, 512 * hh:512 * hh + 512]),
            wvh=bf(wkvf[:, CC:][:, 512 * hh:512 * hh + 512]),
            wcqh=bf(wcqf[:, 512 * hh:512 * hh + 512]),
            wcdh=bf(wcdf[512 * hh:512 * hh + 512, :]),
            wgu0=bf(interleave(np.concatenate(
                [wgu[0][:, IS * c:IS * c + IS],
                 wgu[0][:, I + IS * c:I + IS * c + IS]], 1))),
            wgu1=bf(interleave(np.concatenate(
                [wgu[1][:, IS * c:IS * c + IS],
                 wgu[1][:, I + IS * c:I + IS * c + IS]], 1))),
            wdn0=bf(np.concatenate([wdn[0][IS * c:IS * c + IS],
                                    np.zeros((ISP - IS, H), np.float32)], 0)),
            wdn1=bf(np.concatenate([wdn[1][IS * c:IS * c + IS],
                                    np.zeros((ISP - IS, H), np.float32)], 0)),
        )
        in_maps.append(m)

    trace = bool(int(os.environ.get("KTRACE", "0")))
    res = run_bass_kernel_spmd(nc, in_maps, core_ids=list(range(NC_)),
                               trace=trace,
                               tmpdir=os.environ.get("KTRACE_DIR") or None)
    kernel.last_exec_ns = res.exec_time_ns
    kernel.last_trace = res.instructions_and_trace
    kernel.last_results = res.results
    kernel.last_tslices = tslices
    tot = res.results[0]["y"].astype(np.float64)
    for c in range(1, NC_):
        tot += res.results[c]["y"]
    for c in range(NC_):
        tot[:, tslices[c]] += res.results[c]["h2out"]
    out = np.empty((S, H), np.float32)
    out[perm, :] = tot.T.astype(np.float32)
    return out


# revision 35
# speedup vs baseline: 1.3479x; 1.0178x over previous
"""Trainium2 Bass kernel for nn_CogAgentDecoderLayer (8-core SPMD).

Feature-major activations [feat, tok] in permuted token order
(vis-only | both | lang-only | neither). TP plan per core c
(hh = c//4 head-half, g = c%4 token-group):
  - self-attn: heads split 2/core, causal block-skip, masks only for
    mixed diagonal blocks; dense row-parallel bf16 -> feature-split
    ReduceScatter x2 (RS_a overlaps dense 2nd half).
  - RS token slot T_c = [512*g + 256*hh, +256).
  - cross-attn: KV proj E-sharded + 4-group AllGather (issued early,
    hides behind self-attn); per-core 8 heads x 512 tokens (N=512
    matmuls); softmax denom folded into AV via 65th ones-row; cdense
    half-contraction + pairwise ReduceScatter.
  - MLP: I sliced 688/core, routed by expert; h2n AllGather split by
    feature halves so AG_b hides behind first-half gate_up. y bf16
    partials summed on host.
All matmuls bf16 (fp32 matmul is 2x rows on PE).
"""
import os
import numpy as np
from contextlib import ExitStack
from concourse import bacc, tile, mybir
from concourse.bass_utils import run_bass_kernel_spmd

NC_ = 8
S, E, H, NH, HD = 2048, 2048, 2048, 16, 128
CH, CC, CHD = 1024, 1024, 64
I = 5504
IS = I // NC_          # 688
ISP = 768              # padded to 6*128
EPS = 1e-5
ROPE_BASE = 10000.0
F32 = mybir.dt.float32
BF16 = mybir.dt.bfloat16


def _segs(lo, hi, b0, b1, b2):
    pts = sorted({lo, hi, *[b for b in (b0, b1, b2) if lo < b < hi]})
    out = []
    for s, e in zip(pts, pts[1:]):
        ex = []
        if s < b1:
            ex.append(0)
        if b0 <= s < b2:
            ex.append(1)
        out.append((s, e, ex))
    return out


def _chunks(lo, hi, w):
    out = []
    while lo < hi:
        out.append((lo, min(lo + w, hi)))
        lo += w
    return out


def build_kernel(b0, b1, b2, blocks):
    # blocks: dict (qc, kt) -> ('F', -1) or ('M', mix_idx); absent = skip
    n_mix = max(1, sum(1 for v in blocks.values() if v[0] == 'M'))
    nc = bacc.Bacc("TRN2", target_bir_lowering=False, debug=False,
                   num_devices=NC_)
    din = lambda n, sh, dt: nc.dram_tensor(n, sh, dt, kind="ExternalInput")
    hT = din("hT", [H, S], BF16)
    wqkv0 = din("wqkv0", [H, 768], BF16)
    wqkv1 = din("wqkv1", [H, 768], BF16)
    wd0 = din("wd0", [256, H], BF16)
    wd1 = din("wd1", [256, H], BF16)
    cos2 = din("cos2", [128, S], BF16)
    sin2 = din("sin2", [128, S], BF16)
    rotT = din("rotT", [128, 128], BF16)
    onesb = din("onesb", [128, 128], BF16)
    maskmix = din("maskmix", [n_mix * 128, 512], BF16)
    resid = din("resid", [H, 256], F32)
    encTs = din("encTs", [CH, 512], BF16)
    wkh = din("wkh", [CH, 512], BF16)
    wvh = din("wvh", [CH, 512], BF16)
    wcqh = din("wcqh", [H, 512], BF16)
    wcdh = din("wcdh", [512, H], BF16)
    wgu0 = din("wgu0", [H, 2 * IS], BF16)
    wgu1 = din("wgu1", [H, 2 * IS], BF16)
    wdn0 = din("wdn0", [ISP, H], BF16)
    wdn1 = din("wdn1", [ISP, H], BF16)
    y = nc.dram_tensor("y", [H, S], BF16, kind="ExternalOutput")

    SC = 1.0 / float(np.sqrt(HD))
    CSC = 1.0 / float(np.sqrt(CHD))
    EXP = mybir.ActivationFunctionType.Exp
    SQ = mybir.ActivationFunctionType.Square
    SQRT = mybir.ActivationFunctionType.Sqrt
    SILU = mybir.ActivationFunctionType.Silu
    r128 = lambda ap: ap.rearrange("(c p) n -> p c n", p=128)

    with tile.TileContext(nc) as tc, ExitStack() as top:
        const = top.enter_context(tc.tile_pool(name="const", bufs=1))
        ones_bf = const.tile([128, 128], BF16)
        nc.sync.dma_start(ones_bf[:], onesb.ap()[:])
        rot_sb = const.tile([128, 128], BF16)
        nc.sync.dma_start(rot_sb[:], rotT.ap()[:])
        from concourse.masks import make_identity
        ident = const.tile([128, 128], BF16)
        make_identity(nc, ident[:])
        cos_sb = const.tile([128, S], BF16)
        sin_sb = const.tile([128, S], BF16)
        eps_sb = const.tile([128, 1], F32)
        nc.vector.memset(eps_sb[:], EPS)

        dram = top.enter_context(tc.tile_pool(name="dram", bufs=1, space="DRAM"))
        bounce_a = dram.tile([8 * 1024, 256], BF16)
        bounce_b = dram.tile([8 * 1024, 256], BF16)
        rs_a = dram.tile([1024, 256], BF16)
        rs_b = dram.tile([1024, 256], BF16)
        h1n_bnc = dram.tile([H, 256], BF16)
        h1n_pair = dram.tile([2 * H, 256], BF16)
        kvcon = dram.tile([1024, 520], BF16)
        kvagg = dram.tile([4096, 520], BF16)
        cdpart = dram.tile([2 * H, 256], BF16)
        cdsum = dram.tile([H, 256], BF16)
        h2n_bnc_a = dram.tile([1024, 256], BF16)
        h2n_bnc_b = dram.tile([1024, 256], BF16)
        h2na = dram.tile([8 * 1024, 256], BF16, addr_space="Shared")
        h2nb = dram.tile([8 * 1024, 256], BF16, addr_space="Shared")
        h2out = nc.dram_tensor("h2out", [H, 256], F32, kind="ExternalOutput")

        scrp = top.enter_context(tc.tile_pool(name="scr", bufs=2))

        pABC = top.enter_context(ExitStack())
        qkp = pABC.enter_context(tc.tile_pool(name="qkp", bufs=1))
        qkv_sb = qkp.tile([128, 6, S], BF16)      # q0 q1 k0 k1 v0 v1
        v_sb = qkp.tile([128, 16, 256], BF16)     # token-major v
        ctxp = pABC.enter_context(tc.tile_pool(name="ctxp", bufs=1))
        ctx_sb = ctxp.tile([128, 2, S], BF16)
        hp = pABC.enter_context(ExitStack())
        hpool = hp.enter_context(tc.tile_pool(name="hp", bufs=1))
        h_sb = [hpool.tile([128, 16, 512], BF16, name=f"h{i}")
                for i in range(4)]
        for ci in range(4):
            nc.sync.dma_start(h_sb[ci][:],
                              r128(hT.ap()[:, ci * 512:ci * 512 + 512]))
        nc.sync.dma_start(cos_sb[:], cos2.ap()[:])
        nc.sync.dma_start(sin_sb[:], sin2.ap()[:])

        # ===== phase A: rmsnorm + QKV + rope + vT =====
        with ExitStack() as pA:
            with ExitStack() as pA1:
                nrm = pA1.enter_context(tc.tile_pool(name="nrm", bufs=2))
                nps = pA1.enter_context(tc.tile_pool(name="nps", bufs=2,
                                                     space="PSUM"))
                for ci, (t0, t1) in enumerate(_chunks(0, S, 512)):
                    pss = nps.tile([128, 512], F32, name="pss", tag="pss")
                    for kc in range(16):
                        sq = nrm.tile([128, 512], BF16, name="sq", tag="sq")
                        nc.scalar.activation(sq[:], h_sb[ci][:, kc, :], SQ)
                        nc.tensor.matmul(pss[:], ones_bf[:], sq[:],
                                         start=(kc == 0), stop=(kc == 15))
                    rms = nrm.tile([128, 512], F32, name="rms", tag="rms")
                    nc.scalar.activation(rms[:], pss[:], SQRT,
                                         scale=1.0 / H, bias=eps_sb[:])
                    rinv = nrm.tile([128, 512], F32, name="rinv", tag="rinv")
                    nc.vector.reciprocal(rinv[:], rms[:])
                    for kc in range(16):
                        nc.vector.tensor_mul(h_sb[ci][:, kc, :],
                                             h_sb[ci][:, kc, :], rinv[:])
            with ExitStack() as pA2:
                wp = pA2.enter_context(tc.tile_pool(name="wp", bufs=6))
                mps = pA2.enter_context(tc.tile_pool(name="mps", bufs=2,
                                                     space="PSUM"))
                wtab = {}
                for slot in range(6):
                    for ex, wsrc in ((0, wqkv0), (1, wqkv1)):
                        wt = wp.tile([128, 16, 128], BF16,
                                     name=f"wq{ex}{slot}", tag=f"wq{ex}")
                        nc.sync.dma_start(
                            wt[:], r128(wsrc.ap()[:, slot * 128:slot * 128 + 128]))
                        wtab[(ex, slot)] = wt
                for ci, (t0, t1) in enumerate(_chunks(0, S, 512)):
                    sg = [x for x in _segs(t0, t1, b0, b1, b2) if x[2]]
                    for slot in range(6):
                        if not sg:
                            continue
                        need = sorted({x for _, _, ex in sg for x in ex})
                        pss_ = {}
                        for x in need:
                            ps = mps.tile([128, 512], F32, name=f"qps{x}",
                                          tag=f"qps{x}")
                            for kc in range(16):
                                nc.tensor.matmul(ps[:], wtab[(x, slot)][:, kc, :],
                                                 h_sb[ci][:, kc, :],
                                                 start=(kc == 0), stop=(kc == 15))
                            pss_[x] = ps
                        for s, e, ex in sg:
                            if len(ex) == 1:
                                nc.vector.tensor_copy(qkv_sb[:, slot, s:e],
                                                      pss_[ex[0]][:, s - t0:e - t0])
                            else:
                                nc.vector.tensor_add(qkv_sb[:, slot, s:e],
                                                     pss_[0][:, s - t0:e - t0],
                                                     pss_[1][:, s - t0:e - t0])
                if b2 < S:
                    for slot in range(6):
                        nc.vector.memset(qkv_sb[:, slot, b2:S], 0.0)
                # rope on q,k
                for slot in range(4):
                    for t0, t1 in _chunks(0, S, 512):
                        rp = mps.tile([128, 512], F32, name="rps", tag="qps0")
                        nc.tensor.matmul(rp[:], rot_sb[:],
                                         qkv_sb[:, slot, t0:t1],
                                         start=True, stop=True)
                        c1 = scrp.tile([128, 512], F32, name="ropec", tag="ropec")
                        nc.vector.tensor_mul(c1[:], qkv_sb[:, slot, t0:t1],
                                             cos_sb[:, t0:t1])
                        s1 = scrp.tile([128, 512], F32, name="ropes", tag="ropes")
                        nc.vector.tensor_mul(s1[:], rp[:], sin_sb[:, t0:t1])
                        nc.vector.tensor_add(qkv_sb[:, slot, t0:t1],
                                             c1[:], s1[:])
                # v -> token-major via PE transpose
                for hh in range(2):
                    for tt in range(16):
                        tp = mps.tile([128, 512], BF16, name="tps", tag="qps0")
                        nc.tensor.transpose(
                            tp[:, :128],
                            qkv_sb[:, 4 + hh, tt * 128:tt * 128 + 128],
                            ident[:])
                        nc.vector.tensor_copy(v_sb[:, tt, hh * 128:hh * 128 + 128],
                                              tp[:, :128])

        hp.close()
        # ===== phase KV: E-sharded cross K/V projection + AllGather =====
        with ExitStack() as pK:
            kp_ = pK.enter_context(tc.tile_pool(name="kvp", bufs=1))
            kps = pK.enter_context(tc.tile_pool(name="kvps", bufs=2,
                                                space="PSUM"))
            enc_sb = kp_.tile([128, 8, 512], BF16)
            nc.sync.dma_start(enc_sb[:], r128(encTs.ap()))
            wk_sb = kp_.tile([128, 8, 512], BF16)
            nc.sync.dma_start(wk_sb[:], r128(wkh.ap()))
            wv_sb = kp_.tile([128, 8, 512], BF16)
            nc.sync.dma_start(wv_sb[:], r128(wvh.ap()))
            kcon_sb = kp_.tile([128, 4, 512], BF16)
            vcon_sb = kp_.tile([128, 4, 520], BF16)
            for db in range(4):
                ps = kps.tile([128, 512], F32, name="kcp", tag="kcp")
                for kc in range(8):
                    nc.tensor.matmul(ps[:], wk_sb[:, kc, db * 128:db * 128 + 128],
                                     enc_sb[:, kc, :],
                                     start=(kc == 0), stop=(kc == 7))
                nc.vector.tensor_copy(kcon_sb[:, db, :], ps[:])
            for hl in range(8):
                nc.vector.memset(vcon_sb[:, :, 65 * hl + 64], 1.0)
            for eb in range(4):
                ps = kps.tile([128, 512], F32, name="vcp", tag="kcp")
                for kc in range(8):
                    nc.tensor.matmul(ps[:], enc_sb[:, kc, eb * 128:eb * 128 + 128],
                                     wv_sb[:, kc, :],
                                     start=(kc == 0), stop=(kc == 7))
                for hl in range(8):
                    nc.vector.tensor_copy(
                        vcon_sb[:, eb, 65 * hl:65 * hl + 64],
                        ps[:, 64 * hl:64 * hl + 64])
            nc.sync.dma_start(r128(kvcon[0:512, 0:512]), kcon_sb[:])
            nc.sync.dma_start(r128(kvcon[512:1024, :]), vcon_sb[:])
        nc.gpsimd.collective_compute(
            "AllGather", mybir.AluOpType.bypass,
            replica_groups=[[0, 1, 2, 3], [4, 5, 6, 7]],
            ins=[kvcon.opt()], outs=[kvagg.opt()])

        # ===== phase B: self-attention (block-skip causal) =====
        with ExitStack() as pB:
            ap_ = pB.enter_context(tc.tile_pool(name="ap", bufs=3))
            aps = pB.enter_context(tc.tile_pool(name="aps", bufs=2, space="PSUM"))
            accp = pB.enter_context(tc.tile_pool(name="accp", bufs=2, space="PSUM"))
            mtiles = {}
            for qc in range(4):
                t0 = qc * 512
                live = [kt for kt in range(16) if (qc, kt) in blocks]
                for hh in range(2):
                    pss = accp.tile([128, 512], F32, name="pssum", tag="pssum")
                    psc = accp.tile([128, 512], F32, name="psctx", tag="psctx")
                    for kt in live:
                        cls, mix = blocks[(qc, kt)]
                        mt_ = None
                        if cls == 'M':
                            if hh == 0:
                                mt_ = ap_.tile([128, 512], BF16, name="mt",
                                               tag=f"mt{mix % 4}")
                                nc.sync.dma_start(
                                    mt_[:],
                                    maskmix.ap()[mix * 128:mix * 128 + 128, :])
                                mtiles[mix] = mt_
                            else:
                                mt_ = mtiles[mix]
                        sc = aps.tile([128, 512], F32, name="sc", tag="sc")
                        nc.tensor.matmul(
                            sc[:], qkv_sb[:, 2 + hh, kt * 128:kt * 128 + 128],
                            qkv_sb[:, hh, t0:t0 + 512], start=True, stop=True)
                        if mt_ is not None:
                            nc.vector.tensor_add(sc[:], sc[:], mt_[:])
                        pr = ap_.tile([128, 512], BF16, name="pr", tag="pr")
                        nc.scalar.activation(pr[:], sc[:], EXP, scale=SC)
                        nc.tensor.matmul(pss[:], ones_bf[:], pr[:],
                                         start=(kt == live[0]),
                                         stop=(kt == live[-1]))
                        nc.tensor.matmul(
                            psc[:], v_sb[:, kt, hh * 128:hh * 128 + 128],
                            pr[:], start=(kt == live[0]), stop=(kt == live[-1]))
                    rc = ap_.tile([128, 512], F32, name="rc", tag="rc")
                    nc.vector.reciprocal(rc[:], pss[:])
                    nc.vector.tensor_mul(ctx_sb[:, hh, t0:t0 + 512],
                                         psc[:], rc[:])

        # ===== phase C: dense (routed, bf16) -> feature-split RS =====
        with ExitStack() as pC:
            dwp = pC.enter_context(tc.tile_pool(name="dwp", bufs=1))
            dps = pC.enter_context(tc.tile_pool(name="dps", bufs=2, space="PSUM"))
            dop = pC.enter_context(tc.tile_pool(name="dop", bufs=4))
            dwts = []
            for ex, wsrc in ((0, wd0), (1, wd1)):
                dwt = dwp.tile([128, 2, H], BF16, name=f"dw{ex}", tag=f"dw{ex}")
                nc.sync.dma_start(dwt[:], r128(wsrc.ap()))
                dwts.append(dwt)
            # dram view [p, half, slot*mt, n]: row = half*4096 + c*128 + p
            rb = lambda t: t[:].rearrange("(h c p) n -> p h c n", h=2, p=128)
            bnc_a = rb(bounce_a)
            bnc_b = rb(bounce_b)
            for half, bnc in ((0, bnc_a), (1, bnc_b)):
                for ci in range(4):
                    t0 = ci * 512
                    sg = _segs(t0, t0 + 512, b0, b1, b2)
                    live = [x for x in sg if x[2]]
                    # staging tile [128, 2(half), 8(mt), 256]
                    big = dop.tile([128, 2, 8, 256], BF16, name="dot", tag="dot")
                    for mt in range(8 * half, 8 * half + 8):
                        ot = big[:, :, mt - 8 * half, :]
                        if live:
                            need = sorted({x for _, _, ex in live for x in ex})
                            pss_ = {}
                            for x in need:
                                ps = dps.tile([128, 512], F32, name=f"dpst{x}",
                                              tag=f"dpst{x}")
                                for kc in range(2):
                                    nc.tensor.matmul(
                                        ps[:],
                                        dwts[x][:, kc, mt * 128:mt * 128 + 128],
                                        ctx_sb[:, kc, t0:t0 + 512],
                                        start=(kc == 0), stop=(kc == 1))
                                pss_[x] = ps
                            for s, e, ex in sg:
                                so, eo = s - t0, e - t0
                                dsl = ot[:, so // 256:(eo + 255) // 256, :]
                                if so // 256 == (eo - 1) // 256:
                                    dsl = ot[:, so // 256, so % 256:
                                             so % 256 + (eo - so)]
                                else:
                                    dsl = None
                                if dsl is None:
                                    # crosses the 256 boundary: do per-half
                                    parts = [(so, min(eo, 256)), (max(so, 256), eo)]
                                    parts = [(a, b) for a, b in parts if a < b]
                                else:
                                    parts = [(so, eo)]
                                for a, b in parts:
                                    dv = ot[:, a // 256, a % 256:a % 256 + (b - a)]
                                    if len(ex) == 2:
                                        nc.vector.tensor_add(dv, pss_[0][:, a:b],
                                                             pss_[1][:, a:b])
                                    elif ex:
                                        nc.vector.tensor_copy(dv,
                                                              pss_[ex[0]][:, a:b])
                                    else:
                                        nc.vector.memset(dv, 0.0)
                        else:
                            nc.vector.memset(ot[:], 0.0)
                    for hx in range(2):
                        nc.sync.dma_start(bnc[:, hx, ci * 8:ci * 8 + 8, :],
                                          big[:, hx, :, :])
                if half == 0:
                    nc.gpsimd.collective_compute(
                        "ReduceScatter", mybir.AluOpType.add,
                        replica_groups=[list(range(NC_))],
                        ins=[bounce_a.opt()], outs=[rs_a.opt()])
                else:
                    nc.gpsimd.collective_compute(
                        "ReduceScatter", mybir.AluOpType.add,
                        replica_groups=[list(range(NC_))],
                        ins=[bounce_b.opt()], outs=[rs_b.opt()])
        pABC.close()

        pDP = top.enter_context(ExitStack())
        dp = pDP.enter_context(tc.tile_pool(name="dp", bufs=1))
        h1_sb = dp.tile([128, 16, 256], F32)
        pCC = top.enter_context(ExitStack())
        cctxp = pCC.enter_context(tc.tile_pool(name="cctxp", bufs=1))
        cctx_sb = cctxp.tile([128, 4, 512], BF16)
        pD2E = top.enter_context(ExitStack())
        d2 = pD2E.enter_context(tc.tile_pool(name="d2", bufs=1))
        # cross-attn K/V + wcq loads: kvagg ready mid-B, so these flow on the
        # sync queue during the RS window
        k_sb = d2.tile([128, 4, 2048], BF16)
        for hl in range(8):
            for src_ in range(4):
                nc.sync.dma_start(
                    k_sb[64 * (hl % 2):64 * (hl % 2) + 64, hl // 2,
                         512 * src_:512 * src_ + 512],
                    kvagg[1024 * src_ + 64 * hl:
                          1024 * src_ + 64 * hl + 64, 0:512])
        v_sb2 = d2.tile([128, 16, 520], BF16)
        for src_ in range(4):
            nc.sync.dma_start(
                v_sb2[:, 4 * src_:4 * src_ + 4, :],
                r128(kvagg[1024 * src_ + 512:1024 * src_ + 1024, :]))
        wcq_sb = d2.tile([128, 16, 512], BF16)
        nc.sync.dma_start(wcq_sb[:], r128(wcqh.ap()))

        # ===== phase D1: h1 = rs + resid, rmsnorm, pairwise h1n AG =====
        with ExitStack() as pD1:
            d1 = pD1.enter_context(tc.tile_pool(name="d1", bufs=1))
            d1ps = pD1.enter_context(tc.tile_pool(name="d1ps", bufs=1,
                                                  space="PSUM"))
            re_sb = d1.tile([128, 16, 256], F32)
            nc.sync.dma_start(re_sb[:], r128(resid.ap()))
            rsl_sb = d1.tile([128, 16, 256], BF16)
            nc.sync.dma_start(rsl_sb[:, 0:8, :], r128(rs_a[:]))
            nc.sync.dma_start(rsl_sb[:, 8:16, :], r128(rs_b[:]))
            pss = d1ps.tile([128, 256], F32, name="psd", tag="psd")
            for kc in range(16):
                nc.vector.tensor_add(h1_sb[:, kc, :],
                                     rsl_sb[:, kc, :], re_sb[:, kc, :])
                sq = scrp.tile([128, 256], BF16, name="sqd", tag="sqd")
                nc.scalar.activation(sq[:], h1_sb[:, kc, :], SQ)
                nc.tensor.matmul(pss[:], ones_bf[:], sq[:],
                                 start=(kc == 0), stop=(kc == 15))
            rms1 = d1.tile([128, 256], F32)
            nc.scalar.activation(rms1[:], pss[:], SQRT,
                                 scale=1.0 / H, bias=eps_sb[:])
            rinv = d1.tile([128, 256], F32)
            nc.vector.reciprocal(rinv[:], rms1[:])
            h1n_sb = d1.tile([128, 16, 256], BF16)
            for kc in range(16):
                nc.vector.tensor_mul(h1n_sb[:, kc, :],
                                     h1_sb[:, kc, :], rinv[:])
            nc.sync.dma_start(r128(h1n_bnc[:]), h1n_sb[:])
        nc.gpsimd.collective_compute(
            "AllGather", mybir.AluOpType.bypass,
            replica_groups=[[0, 4], [1, 5], [2, 6], [3, 7]],
            ins=[h1n_bnc.opt()], outs=[h1n_pair.opt()])

        # ===== phase D2: cross-attn, 8 heads x 512 tokens =====
        with ExitStack() as pD2:
            h1n5 = d2.tile([128, 16, 512], BF16)
            for r in range(2):
                nc.sync.dma_start(h1n5[:, :, 256 * r:256 * r + 256],
                                  r128(h1n_pair[H * r:H * (r + 1), :]))
            cq_sb = d2.tile([128, 4, 512], BF16)
            d2ps = pD2.enter_context(tc.tile_pool(name="d2ps", bufs=2,
                                                  space="PSUM"))
            for sl in range(4):
                ps = d2ps.tile([128, 512], F32, name="cqp", tag="cqp")
                for kc in range(16):
                    nc.tensor.matmul(ps[:],
                                     wcq_sb[:, kc, sl * 128:sl * 128 + 128],
                                     h1n5[:, kc, :],
                                     start=(kc == 0), stop=(kc == 15))
                nc.vector.tensor_copy(cq_sb[:, sl, :], ps[:])
            with ExitStack() as pD3:
                cap = pD3.enter_context(tc.tile_pool(name="cap", bufs=3))
                caps = pD3.enter_context(tc.tile_pool(name="caps", bufs=2,
                                                      space="PSUM"))
                cacc = pD3.enter_context(tc.tile_pool(name="cacc", bufs=1,
                                                      space="PSUM"))
                for wave in range(2):
                    psc = [cacc.tile([65, 512], F32, name=f"cps{j}", tag=f"cps{j}")
                           for j in range(4)]
                    for kt in range(16):
                        for j in range(4):
                            hl = 4 * wave + j
                            sc = caps.tile([128, 512], F32, name="csc", tag="csc")
                            nc.tensor.matmul(
                                sc[:],
                                k_sb[64 * (hl % 2):64 * (hl % 2) + 64, hl // 2,
                                     kt * 128:kt * 128 + 128],
                                cq_sb[64 * (hl % 2):64 * (hl % 2) + 64,
                                      hl // 2, :],
                                start=True, stop=True)
                            pr = cap.tile([128, 512], BF16, name="cpr", tag="cpr")
                            nc.scalar.activation(pr[:], sc[:], EXP, scale=CSC)
                            nc.tensor.matmul(
                                psc[j][:],
                                v_sb2[:, kt, 65 * hl:65 * hl + 65],
                                pr[:], start=(kt == 0), stop=(kt == 15))
                    for j in range(4):
                        hl = 4 * wave + j
                        dnf = cap.tile([1, 512], F32, name="dnf", tag="dnf")
                        nc.vector.tensor_copy(dnf[:], psc[j][64:65, :])
                        rcf = cap.tile([1, 512], F32, name="rcf", tag="rcf")
                        nc.vector.reciprocal(rcf[:], dnf[:])
                        rcb = cap.tile([1, 512], BF16, name="rcb", tag="rcb")
                        nc.vector.tensor_copy(rcb[:], rcf[:])
                        dnb = caps.tile([64, 512], F32, name="dnb", tag="csc")
                        nc.tensor.matmul(dnb[:], ones_bf[0:1, 0:64], rcb[:],
                                         start=True, stop=True)
                        rbf = cap.tile([64, 512], F32, name="rbf", tag="rbf")
                        nc.vector.tensor_copy(rbf[:], dnb[:])
                        po = 64 * (hl % 2)
                        nc.vector.tensor_mul(cctx_sb[po:po + 64, hl // 2, :],
                                             psc[j][0:64, :], rbf[:])

        pD2E.close()
        # ===== phase D2b: cdense half-contraction -> pairwise RS =====
        with ExitStack() as pD4:
            d4 = pD4.enter_context(tc.tile_pool(name="d4", bufs=1))
            d4o = pD4.enter_context(tc.tile_pool(name="d4o", bufs=4))
            wcd_sb = d4.tile([128, 4, H], BF16)
            nc.sync.dma_start(wcd_sb[:], r128(wcdh.ap()))
            d4ps = pD4.enter_context(tc.tile_pool(name="d4ps", bufs=2,
                                                  space="PSUM"))
            cdp_t = cdpart[:].rearrange("(h c p) n -> p h c n", h=2, p=128)
            for mg in range(2):
                big = d4o.tile([128, 2, 8, 256], BF16, name="cdo", tag="cdo")
                for mj in range(8):
                    mt = mg * 8 + mj
                    ps = d4ps.tile([128, 512], F32, name="cdp", tag="cdp")
                    for kc in range(4):
                        nc.tensor.matmul(ps[:],
                                         wcd_sb[:, kc, mt * 128:mt * 128 + 128],
                                         cctx_sb[:, kc, :],
                                         start=(kc == 0), stop=(kc == 3))
                    nc.vector.tensor_copy(big[:, 0, mj, :], ps[:, 0:256])
                    nc.vector.tensor_copy(big[:, 1, mj, :], ps[:, 256:512])
                for hx in range(2):
                    nc.sync.dma_start(cdp_t[:, hx, mg * 8:mg * 8 + 8, :],
                                      big[:, hx, :, :])
        pCC.close()
        nc.gpsimd.collective_compute(
            "ReduceScatter", mybir.AluOpType.add,
            replica_groups=[[0, 4], [1, 5], [2, 6], [3, 7]],
            ins=[cdpart.opt()], outs=[cdsum.opt()])

        # ===== phase D3: h2 = h1 + cdsum, rmsnorm, split AG =====
        with ExitStack() as pD5:
            d5 = pD5.enter_context(tc.tile_pool(name="d5", bufs=1))
            cds_sb = d5.tile([128, 16, 256], BF16)
            nc.sync.dma_start(cds_sb[:], r128(cdsum[:]))
            h2_sb = d5.tile([128, 16, 256], F32)
            d5ps = pD5.enter_context(tc.tile_pool(name="d5ps", bufs=1,
                                                  space="PSUM"))
            pss2 = d5ps.tile([128, 256], F32, name="psd2", tag="psd2")
            for kc in range(16):
                nc.vector.tensor_add(h2_sb[:, kc, :], cds_sb[:, kc, :],
                                     h1_sb[:, kc, :])
                sq = scrp.tile([128, 256], BF16, name="sqd2", tag="sqd")
                nc.scalar.activation(sq[:], h2_sb[:, kc, :], SQ)
                nc.tensor.matmul(pss2[:], ones_bf[:], sq[:],
                                 start=(kc == 0), stop=(kc == 15))
            nc.sync.dma_start(r128(h2out.ap()), h2_sb[:])
            rms2 = d5.tile([128, 256], F32)
            nc.scalar.activation(rms2[:], pss2[:], SQRT,
                                 scale=1.0 / H, bias=eps_sb[:])
            rinv2 = d5.tile([128, 256], F32)
            nc.vector.reciprocal(rinv2[:], rms2[:])
            h2n_sb = d5.tile([128, 16, 256], BF16)
            for kc in range(16):
                nc.vector.tensor_mul(h2n_sb[:, kc, :],
                                     h2_sb[:, kc, :], rinv2[:])
            nc.sync.dma_start(r128(h2n_bnc_a[:]), h2n_sb[:, 0:8, :])
            nc.sync.dma_start(r128(h2n_bnc_b[:]), h2n_sb[:, 8:16, :])
        pDP.close()
        # MLP expert-0 weight prefetch: no deps, so these flow on the sync
        # queue during pairRS/D3/AG. Expert-1 tiles reuse the same tags, so
        # their DMAs are issued inside phase F (a tag-reuse wait here would
        # stall the sync queue pre-AG).
        fw = top.enter_context(tc.tile_pool(name="fw", bufs=1))

        def load_mlp_w(ex, gsrc, dsrc):
            dn_t = fw.tile([128, 6, H], BF16, name=f"dn{ex}", tag="dn")
            nc.sync.dma_start(dn_t[:], r128(dsrc.ap()))
            gws = []
            for pi in range(6):
                gw = 128 if pi < 5 else 48
                gwt = fw.tile([128, 16, 256], BF16,
                              name=f"guw{ex}{pi}", tag=f"guw{pi}")
                nc.sync.dma_start(
                    gwt[:, :, :2 * gw],
                    r128(gsrc.ap()[:, pi * 256:pi * 256 + 2 * gw]))
                gws.append(gwt)
            return dn_t, gws

        dn_ts = {}
        gw_ts = {}
        dn_ts[0], gw_ts[0] = load_mlp_w(0, wgu0, wdn0)
        nc.gpsimd.collective_compute(
            "AllGather", mybir.AluOpType.bypass,
            replica_groups=[list(range(NC_))],
            ins=[h2n_bnc_a.opt()], outs=[h2na.opt()])
        nc.gpsimd.collective_compute(
            "AllGather", mybir.AluOpType.bypass,
            replica_groups=[list(range(NC_))],
            ins=[h2n_bnc_b.opt()], outs=[h2nb.opt()])

        # ===== phase F: MLP (routed by expert ranges, bf16) =====
        with ExitStack() as pF:
            fp = pF.enter_context(tc.tile_pool(name="fp", bufs=1))
            hn_sb = fp.tile([128, 16, S], BF16)
            for r in (0, 4, 1, 5, 2, 6, 3, 7):
                tr = 512 * (r % 4) + 256 * (r // 4)
                nc.sync.dma_start(hn_sb[:, 0:8, tr:tr + 256],
                                  r128(h2na[r * 1024:(r + 1) * 1024, :]))
            for r in (0, 4, 1, 5, 2, 6, 3, 7):
                tr = 512 * (r % 4) + 256 * (r // 4)
                nc.sync.dma_start(hn_sb[:, 8:16, tr:tr + 256],
                                  r128(h2nb[r * 1024:(r + 1) * 1024, :]))
            fps = pF.enter_context(tc.tile_pool(name="fps", bufs=2, space="PSUM"))
            fpd = pF.enter_context(tc.tile_pool(name="fpd", bufs=2, space="PSUM"))
            fac = pF.enter_context(tc.tile_pool(name="fac", bufs=2))
            fout = pF.enter_context(tc.tile_pool(name="fout", bufs=4))
            for ex, (lo, hi) in ((0, (0, b1)), (1, (b1, S))):
                if ex not in dn_ts:
                    dn_ts[ex], gw_ts[ex] = load_mlp_w(ex, (wgu0, wgu1)[ex],
                                                      (wdn0, wdn1)[ex])
                dn_t = dn_ts[ex]
                gwts = gw_ts[ex]
                for a0 in range(0, S, 512):
                    c0, c1 = max(a0, lo), min(a0 + 512, hi)
                    if c0 >= c1:
                        continue
                    t0_, W = a0, 512
                    eo, ew = c0 - a0, c1 - c0
                    act = fac.tile([128, 6, 512], BF16, name="act", tag="act")
                    for pi in range(6):
                        gw = 128 if pi < 5 else 48
                        gwt = gwts[pi]
                        pg = fps.tile([128, 512], F32, name="pg", tag="pg")
                        pu = fps.tile([128, 512], F32, name="pu", tag="pu")
                        for kc in range(16):
                            nc.tensor.matmul(pg[:gw, :W], gwt[:, kc, :gw],
                                             hn_sb[:, kc, t0_:t0_ + 512],
                                             start=(kc == 0), stop=(kc == 15))
                            nc.tensor.matmul(pu[:gw, :W], gwt[:, kc, gw:2 * gw],
                                             hn_sb[:, kc, t0_:t0_ + 512],
                                             start=(kc == 0), stop=(kc == 15))
                        gs = scrp.tile([128, 512], F32, name="gs", tag="gs")
                        nc.scalar.activation(gs[:gw, :W], pg[:gw, :W], SILU)
                        nc.vector.tensor_mul(act[:gw, pi, :W],
                                             gs[:gw, :W], pu[:gw, :W])
                    for mt in range(16):
                        pd = fpd.tile([128, 512], F32, name="pd", tag="pd")
                        for pi in range(6):
                            kw = 128 if pi < 5 else 48
                            nc.tensor.matmul(
                                pd[:, :W],
                                dn_t[:kw, pi, mt * 128:mt * 128 + 128],
                                act[:kw, pi, :W],
                                start=(pi == 0), stop=(pi == 5))
                        ot = fout.tile([128, 512], BF16, name="fot", tag="fot")
                        nc.vector.tensor_copy(ot[:, eo:eo + ew], pd[:, eo:eo + ew])
                        nc.sync.dma_start(
                            y.ap()[mt * 128:mt * 128 + 128, c0:c1],
                            ot[:, eo:eo + ew])
    nc.compile()
    return nc


_CACHE = {}


def kernel(**inputs):
    import ml_dtypes
    vm = np.asarray(inputs["vision_token_ids"]).astype(bool)
    lm = np.asarray(inputs["language_token_ids"]).astype(bool)
    g0 = np.where(vm & ~lm)[0]; g1 = np.where(vm & lm)[0]
    g2 = np.where(~vm & lm)[0]; g3 = np.where(~vm & ~lm)[0]
    perm = np.concatenate([g0, g1, g2, g3])
    b0 = len(g0); b1 = b0 + len(g1); b2 = b1 + len(g2)

    f32 = lambda x: np.ascontiguousarray(np.asarray(x, np.float32))
    bf = lambda x: np.ascontiguousarray(np.asarray(x).astype(ml_dtypes.bfloat16))
    pos = np.asarray(inputs["positions"]).astype(np.float32)
    half = HD // 2
    inv_freq = 1.0 / (ROPE_BASE ** (np.arange(half, dtype=np.float32) / half))
    fr = pos[:, None] * inv_freq[None, :]
    cos2 = np.concatenate([np.cos(fr)] * 2, 1).T[:, perm]
    sin2 = np.concatenate([np.sin(fr)] * 2, 1).T[:, perm]
    rot = np.zeros((HD, HD), np.float32)
    rot[np.arange(half), np.arange(half) + half] = -1.0
    rot[np.arange(half) + half, np.arange(half)] = 1.0
    op = np.asarray(inputs["positions"])[perm].astype(np.int64)

    # block classification: rows=keys [128kt,+128), cols=queries [512qc,+512)
    blocks = {}
    mix_tiles = []
    for qc in range(4):
        opq = op[512 * qc:512 * qc + 512]
        for kt in range(16):
            opk = op[128 * kt:128 * kt + 128]
            if opq.min() >= opk.max():
                blocks[(qc, kt)] = ('F', -1)
            elif opq.max() < opk.min():
                pass  # skip
            else:
                blocks[(qc, kt)] = ('M', len(mix_tiles))
                mix_tiles.append(
                    np.where(opq[None, :] >= opk[:, None], 0.0, -30000.0))
    maskmix = (np.concatenate(mix_tiles, 0) if mix_tiles
               else np.zeros((128, 512), np.float32))

    wln_in = f32(inputs["w_ln_in"])[:, None]
    wln_pa = f32(inputs["w_ln_post_attn"])[:, None]
    wln_pc = f32(inputs["w_ln_post_cross"])[:, None]
    wqkv = [f32(inputs["w_vis_qkv"]) * wln_in, f32(inputs["w_lang_qkv"]) * wln_in]
    wd = [f32(inputs["w_vis_dense"]), f32(inputs["w_lang_dense"])]
    wgu = [f32(inputs["w_vis_gate_up"]) * wln_pc,
           f32(inputs["w_lang_gate_up"]) * wln_pc]
    wdn = [f32(inputs["w_vis_down"]), f32(inputs["w_lang_down"])]
    wkvf = f32(inputs["w_cross_kv"])
    wcqf = f32(inputs["w_cross_q"]) * wln_pa
    wcdf = f32(inputs["w_cross_dense"])
    encTf = f32(inputs["encoder_embeds"]).T
    hTp = f32(inputs["hidden_states"]).T[:, perm].copy()

    def interleave(w):  # w [H, 2*IS] = [gate | up]
        cols = []
        for i in range(5):
            cols.append(w[:, 128 * i:128 * i + 128])
            cols.append(w[:, IS + 128 * i:IS + 128 * i + 128])
        cols.append(w[:, 640:IS]); cols.append(w[:, IS + 640:2 * IS])
        return np.ascontiguousarray(np.concatenate(cols, 1))

    bsig = tuple(sorted((k, v[0]) for k, v in blocks.items()))
    key = (b0, b1, b2, bsig)
    if key not in _CACHE:
        _CACHE.clear()
        _CACHE[key] = build_kernel(b0, b1, b2, blocks)
    nc = _CACHE[key]

    in_maps = []
    tslices = []
    for c in range(NC_):
        hh, g = c // 4, c % 4
        ts = slice(512 * g + 256 * hh, 512 * g + 256 * hh + 256)
        tslices.append(ts)
        qs = slice(256 * c, 256 * c + 256)
        m = dict(
            hT=bf(hTp),
            wqkv0=bf(np.concatenate([wqkv[0][:, qs], wqkv[0][:, H:][:, qs],
                                     wqkv[0][:, 2 * H:][:, qs]], 1)),
            wqkv1=bf(np.concatenate([wqkv[1][:, qs], wqkv[1][:, H:][:, qs],
                                     wqkv[1][:, 2 * H:][:, qs]], 1)),
            wd0=bf(wd[0][qs]), wd1=bf(wd[1][qs]),
            cos2=bf(cos2), sin2=bf(sin2), rotT=bf(rot.T),
            onesb=np.ones((128, 128), ml_dtypes.bfloat16),
            maskmix=bf(maskmix),
            resid=hTp[:, ts].copy(),
            encTs=bf(encTf[:, 512 * g:512 * g + 512]),
            wkh=bf(wkvf[:, :CC][:, 512 * hh:512 * hh + 512]),
            wvh=bf(wkvf[:, CC:][:, 512 * hh:512 * hh + 512]),
            wcqh=bf(wcqf[:, 512 * hh:512 * hh + 512]),
            wcdh=bf(wcdf[512 * hh:512 * hh + 512, :]),
            wgu0=bf(interleave(np.concatenate(
                [wgu[0][:, IS * c:IS * c + IS],
                 wgu[0][:, I + IS * c:I + IS * c + IS]], 1))),
            wgu1=bf(interleave(np.concatenate(
                [wgu[1][:, IS * c:IS * c + IS],
                 wgu[1][:, I + IS * c:I + IS * c + IS]], 1))),
            wdn0=bf(np.concatenate([wdn[0][IS * c:IS * c + IS],
                                    np.zeros((ISP - IS, H), np.float32)], 0)),
            wdn1=bf(np.concatenate([wdn[1][IS * c:IS * c + IS],
                                    np.zeros((ISP - IS, H), np.float32)], 0)),
        )
        in_maps.append(m)

    trace = bool(int(os.environ.get("KTRACE", "0")))
    res = run_bass_kernel_spmd(nc, in_maps, core_ids=list(range(NC_)),
                               trace=trace,
                               tmpdir=os.environ.get("KTRACE_DIR") or None)
    kernel.last_exec_ns = res.exec_time_ns
    kernel.last_trace = res.instructions_and_trace
    kernel.last_results = res.results
    kernel.last_tslices = tslices
    tot = res.results[0]["y"].astype(np.float64)
    for c in range(1, NC_):
        tot += res.results[c]["y"]
    for c in range(NC_):
        tot[:, tslices[c]] += res.results[c]["h2out"]
    out = np.empty((S, H), np.float32)
    out[perm, :] = tot.T.astype(np.float32)
    return out


# revision 37
# speedup vs baseline: 1.3554x; 1.0056x over previous
"""Trainium2 Bass kernel for nn_CogAgentDecoderLayer (8-core SPMD).

Feature-major activations [feat, tok] in permuted token order
(vis-only | both | lang-only | neither). TP plan per core c
(hh = c//4 head-half, g = c%4 token-group):
  - self-attn: heads split 2/core, causal block-skip, masks only for
    mixed diagonal blocks; dense row-parallel bf16 -> feature-split
    ReduceScatter x2 (RS_a overlaps dense 2nd half).
  - RS token slot T_c = [512*g + 256*hh, +256).
  - cross-attn: KV proj E-sharded + 4-group AllGather (issued early,
    hides behind self-attn); per-core 8 heads x 512 tokens (N=512
    matmuls); softmax denom folded into AV via 65th ones-row; cdense
    half-contraction + pairwise ReduceScatter.
  - MLP: I sliced 688/core, routed by expert; h2n AllGather split by
    feature halves so AG_b hides behind first-half gate_up. y bf16
    partials summed on host.
All matmuls bf16 (fp32 matmul is 2x rows on PE).
"""
import os
import numpy as np
from contextlib import ExitStack
from concourse import bacc, tile, mybir
from concourse.bass_utils import run_bass_kernel_spmd

NC_ = 8
S, E, H, NH, HD = 2048, 2048, 2048, 16, 128
CH, CC, CHD = 1024, 1024, 64
I = 5504
IS = I // NC_          # 688
ISP = 768              # padded to 6*128
EPS = 1e-5
ROPE_BASE = 10000.0
F32 = mybir.dt.float32
BF16 = mybir.dt.bfloat16


def _segs(lo, hi, b0, b1, b2):
    pts = sorted({lo, hi, *[b for b in (b0, b1, b2) if lo < b < hi]})
    out = []
    for s, e in zip(pts, pts[1:]):
        ex = []
        if s < b1:
            ex.append(0)
        if b0 <= s < b2:
            ex.append(1)
        out.append((s, e, ex))
    return out


def _chunks(lo, hi, w):
    out = []
    while lo < hi:
        out.append((lo, min(lo + w, hi)))
        lo += w
    return out


def build_kernel(b0, b1, b2, blocks):
    # blocks: dict (qc, kt) -> ('F', -1) or ('M', mix_idx); absent = skip
    n_mix = max(1, sum(1 for v in blocks.values() if v[0] == 'M'))
    nc = bacc.Bacc("TRN2", target_bir_lowering=False, debug=False,
                   num_devices=NC_)
    din = lambda n, sh, dt: nc.dram_tensor(n, sh, dt, kind="ExternalInput")
    hT = din("hT", [H, S], BF16)
    wqkv0 = din("wqkv0", [H, 768], BF16)
    wqkv1 = din("wqkv1", [H, 768], BF16)
    wd0 = din("wd0", [256, H], BF16)
    wd1 = din("wd1", [256, H], BF16)
    cos2 = din("cos2", [128, S], BF16)
    sin2 = din("sin2", [128, S], BF16)
    rotT = din("rotT", [128, 128], BF16)
    onesb = din("onesb", [128, 128], BF16)
    maskmix = din("maskmix", [n_mix * 128, 512], BF16)
    resid = din("resid", [H, 256], F32)
    encTs = din("encTs", [CH, 512], BF16)
    wkh = din("wkh", [CH, 512], BF16)
    wvh = din("wvh", [CH, 512], BF16)
    wcqh = din("wcqh", [H, 512], BF16)
    wcdh = din("wcdh", [512, H], BF16)
    wgu0 = din("wgu0", [H, 2 * IS], BF16)
    wgu1 = din("wgu1", [H, 2 * IS], BF16)
    wdn0 = din("wdn0", [ISP, H], BF16)
    wdn1 = din("wdn1", [ISP, H], BF16)
    y = nc.dram_tensor("y", [H, S], BF16, kind="ExternalOutput")

    SC = 1.0 / float(np.sqrt(HD))
    CSC = 1.0 / float(np.sqrt(CHD))
    EXP = mybir.ActivationFunctionType.Exp
    SQ = mybir.ActivationFunctionType.Square
    SQRT = mybir.ActivationFunctionType.Sqrt
    SILU = mybir.ActivationFunctionType.Silu
    r128 = lambda ap: ap.rearrange("(c p) n -> p c n", p=128)

    with tile.TileContext(nc) as tc, ExitStack() as top:
        const = top.enter_context(tc.tile_pool(name="const", bufs=1))
        ones_bf = const.tile([128, 128], BF16)
        nc.sync.dma_start(ones_bf[:], onesb.ap()[:])
        rot_sb = const.tile([128, 128], BF16)
        nc.sync.dma_start(rot_sb[:], rotT.ap()[:])
        from concourse.masks import make_identity
        ident = const.tile([128, 128], BF16)
        make_identity(nc, ident[:])
        cos_sb = const.tile([128, S], BF16)
        sin_sb = const.tile([128, S], BF16)
        eps_sb = const.tile([128, 1], F32)
        nc.vector.memset(eps_sb[:], EPS)

        dram = top.enter_context(tc.tile_pool(name="dram", bufs=1, space="DRAM"))
        bounce_a = dram.tile([8 * 1024, 256], BF16)
        bounce_b = dram.tile([8 * 1024, 256], BF16)
        rs_a = dram.tile([1024, 256], BF16)
        rs_b = dram.tile([1024, 256], BF16)
        h1n_bnc = dram.tile([H, 256], BF16)
        h1n_pair = dram.tile([2 * H, 256], BF16)
        kvcon = dram.tile([1024, 520], BF16)
        kvagg = dram.tile([4096, 520], BF16)
        cdpart = dram.tile([2 * H, 256], BF16)
        cdsum = dram.tile([H, 256], BF16)
        h2n_bnc = dram.tile([H, 256], BF16)
        h2n_all = dram.tile([8 * H, 256], BF16, addr_space="Shared")
        h2out = nc.dram_tensor("h2out", [H, 256], F32, kind="ExternalOutput")

        scrp = top.enter_context(tc.tile_pool(name="scr", bufs=2))

        pABC = top.enter_context(ExitStack())
        qkp = pABC.enter_context(tc.tile_pool(name="qkp", bufs=1))
        qkv_sb = qkp.tile([128, 6, S], BF16)      # q0 q1 k0 k1 v0 v1
        v_sb = qkp.tile([128, 16, 256], BF16)     # token-major v
        ctxp = pABC.enter_context(tc.tile_pool(name="ctxp", bufs=1))
        ctx_sb = ctxp.tile([128, 2, S], BF16)
        wq_pre = pABC.enter_context(tc.tile_pool(name="wqpre", bufs=1))
        hp = pABC.enter_context(ExitStack())
        hpool = hp.enter_context(tc.tile_pool(name="hp", bufs=1))
        h_sb = [hpool.tile([128, 16, 512], BF16, name=f"h{i}")
                for i in range(4)]
        wtab = {}
        def _load_wq(slot):
            for ex, wsrc in ((0, wqkv0), (1, wqkv1)):
                wt = wq_pre.tile([128, 16, 128], BF16, name=f"wq{ex}{slot}")
                nc.sync.dma_start(
                    wt[:], r128(wsrc.ap()[:, slot * 128:slot * 128 + 128]))
                wtab[(ex, slot)] = wt
        _load_wq(0)
        for ci in range(4):
            nc.sync.dma_start(h_sb[ci][:],
                              r128(hT.ap()[:, ci * 512:ci * 512 + 512]))
            _load_wq(ci + 1)
        _load_wq(5)
        nc.sync.dma_start(cos_sb[:], cos2.ap()[:])
        nc.sync.dma_start(sin_sb[:], sin2.ap()[:])

        # ===== phase A: rmsnorm + QKV + rope + vT =====
        with ExitStack() as pA:
            with ExitStack() as pA1:
                nrm = pA1.enter_context(tc.tile_pool(name="nrm", bufs=2))
                nps = pA1.enter_context(tc.tile_pool(name="nps", bufs=2,
                                                     space="PSUM"))
                for ci, (t0, t1) in enumerate(_chunks(0, S, 512)):
                    pss = nps.tile([128, 512], F32, name="pss", tag="pss")
                    for kc in range(16):
                        sq = nrm.tile([128, 512], BF16, name="sq", tag="sq")
                        nc.scalar.activation(sq[:], h_sb[ci][:, kc, :], SQ)
                        nc.tensor.matmul(pss[:], ones_bf[:], sq[:],
                                         start=(kc == 0), stop=(kc == 15))
                    rms = nrm.tile([128, 512], F32, name="rms", tag="rms")
                    nc.scalar.activation(rms[:], pss[:], SQRT,
                                         scale=1.0 / H, bias=eps_sb[:])
                    rinv = nrm.tile([128, 512], F32, name="rinv", tag="rinv")
                    nc.vector.reciprocal(rinv[:], rms[:])
                    for kc in range(16):
                        nc.vector.tensor_mul(h_sb[ci][:, kc, :],
                                             h_sb[ci][:, kc, :], rinv[:])
            with ExitStack() as pA2:
                mps = pA2.enter_context(tc.tile_pool(name="mps", bufs=2,
                                                     space="PSUM"))
                for ci, (t0, t1) in enumerate(_chunks(0, S, 512)):
                    sg = [x for x in _segs(t0, t1, b0, b1, b2) if x[2]]
                    for slot in range(6):
                        if not sg:
                            continue
                        need = sorted({x for _, _, ex in sg for x in ex})
                        pss_ = {}
                        for x in need:
                            ps = mps.tile([128, 512], F32, name=f"qps{x}",
                                          tag=f"qps{x}")
                            for kc in range(16):
                                nc.tensor.matmul(ps[:], wtab[(x, slot)][:, kc, :],
                                                 h_sb[ci][:, kc, :],
                                                 start=(kc == 0), stop=(kc == 15))
                            pss_[x] = ps
                        for s, e, ex in sg:
                            if len(ex) == 1:
                                nc.vector.tensor_copy(qkv_sb[:, slot, s:e],
                                                      pss_[ex[0]][:, s - t0:e - t0])
                            else:
                                nc.vector.tensor_add(qkv_sb[:, slot, s:e],
                                                     pss_[0][:, s - t0:e - t0],
                                                     pss_[1][:, s - t0:e - t0])
                if b2 < S:
                    for slot in range(6):
                        nc.vector.memset(qkv_sb[:, slot, b2:S], 0.0)
                # rope on q,k
                for slot in range(4):
                    for t0, t1 in _chunks(0, S, 512):
                        rp = mps.tile([128, 512], F32, name="rps", tag="qps0")
                        nc.tensor.matmul(rp[:], rot_sb[:],
                                         qkv_sb[:, slot, t0:t1],
                                         start=True, stop=True)
                        c1 = scrp.tile([128, 512], F32, name="ropec", tag="ropec")
                        nc.vector.tensor_mul(c1[:], qkv_sb[:, slot, t0:t1],
                                             cos_sb[:, t0:t1])
                        s1 = scrp.tile([128, 512], F32, name="ropes", tag="ropes")
                        nc.vector.tensor_mul(s1[:], rp[:], sin_sb[:, t0:t1])
                        nc.vector.tensor_add(qkv_sb[:, slot, t0:t1],
                                             c1[:], s1[:])
                # v -> token-major via PE transpose
                for hh in range(2):
                    for tt in range(16):
                        tp = mps.tile([128, 512], BF16, name="tps", tag="qps0")
                        nc.tensor.transpose(
                            tp[:, :128],
                            qkv_sb[:, 4 + hh, tt * 128:tt * 128 + 128],
                            ident[:])
                        nc.vector.tensor_copy(v_sb[:, tt, hh * 128:hh * 128 + 128],
                                              tp[:, :128])

        hp.close()
        # ===== phase KV: E-sharded cross K/V projection + AllGather =====
        with ExitStack() as pK:
            kp_ = pK.enter_context(tc.tile_pool(name="kvp", bufs=1))
            kps = pK.enter_context(tc.tile_pool(name="kvps", bufs=2,
                                                space="PSUM"))
            enc_sb = kp_.tile([128, 8, 512], BF16)
            nc.sync.dma_start(enc_sb[:], r128(encTs.ap()))
            wk_sb = kp_.tile([128, 8, 512], BF16)
            nc.sync.dma_start(wk_sb[:], r128(wkh.ap()))
            wv_sb = kp_.tile([128, 8, 512], BF16)
            nc.sync.dma_start(wv_sb[:], r128(wvh.ap()))
            kcon_sb = kp_.tile([128, 4, 512], BF16)
            vcon_sb = kp_.tile([128, 4, 520], BF16)
            for db in range(4):
                ps = kps.tile([128, 512], F32, name="kcp", tag="kcp")
                for kc in range(8):
                    nc.tensor.matmul(ps[:], wk_sb[:, kc, db * 128:db * 128 + 128],
                                     enc_sb[:, kc, :],
                                     start=(kc == 0), stop=(kc == 7))
                nc.vector.tensor_copy(kcon_sb[:, db, :], ps[:])
            for hl in range(8):
                nc.vector.memset(vcon_sb[:, :, 65 * hl + 64], 1.0)
            for eb in range(4):
                ps = kps.tile([128, 512], F32, name="vcp", tag="kcp")
                for kc in range(8):
                    nc.tensor.matmul(ps[:], enc_sb[:, kc, eb * 128:eb * 128 + 128],
                                     wv_sb[:, kc, :],
                                     start=(kc == 0), stop=(kc == 7))
                for hl in range(8):
                    nc.vector.tensor_copy(
                        vcon_sb[:, eb, 65 * hl:65 * hl + 64],
                        ps[:, 64 * hl:64 * hl + 64])
            nc.sync.dma_start(r128(kvcon[0:512, 0:512]), kcon_sb[:])
            nc.sync.dma_start(r128(kvcon[512:1024, :]), vcon_sb[:])
        nc.gpsimd.collective_compute(
            "AllGather", mybir.AluOpType.bypass,
            replica_groups=[[0, 1, 2, 3], [4, 5, 6, 7]],
            ins=[kvcon.opt()], outs=[kvagg.opt()])

        # ===== phase B: self-attention (block-skip causal) =====
        with ExitStack() as pB:
            ap_ = pB.enter_context(tc.tile_pool(name="ap", bufs=3))
            aps = pB.enter_context(tc.tile_pool(name="aps", bufs=2, space="PSUM"))
            accp = pB.enter_context(tc.tile_pool(name="accp", bufs=2, space="PSUM"))
            mtiles = {}
            for qc in range(4):
                t0 = qc * 512
                live = [kt for kt in range(16) if (qc, kt) in blocks]
                for hh in range(2):
                    pss = accp.tile([128, 512], F32, name="pssum", tag="pssum")
                    psc = accp.tile([128, 512], F32, name="psctx", tag="psctx")
                    for kt in live:
                        cls, mix = blocks[(qc, kt)]
                        mt_ = None
                        if cls == 'M':
                            if hh == 0:
                                mt_ = ap_.tile([128, 512], BF16, name="mt",
                                               tag=f"mt{mix % 4}")
                                nc.sync.dma_start(
                                    mt_[:],
                                    maskmix.ap()[mix * 128:mix * 128 + 128, :])
                                mtiles[mix] = mt_
                            else:
                                mt_ = mtiles[mix]
                        sc = aps.tile([128, 512], F32, name="sc", tag="sc")
                        nc.tensor.matmul(
                            sc[:], qkv_sb[:, 2 + hh, kt * 128:kt * 128 + 128],
                            qkv_sb[:, hh, t0:t0 + 512], start=True, stop=True)
                        if mt_ is not None:
                            nc.vector.tensor_add(sc[:], sc[:], mt_[:])
                        pr = ap_.tile([128, 512], BF16, name="pr", tag="pr")
                        nc.scalar.activation(pr[:], sc[:], EXP, scale=SC)
                        nc.tensor.matmul(pss[:], ones_bf[:], pr[:],
                                         start=(kt == live[0]),
                                         stop=(kt == live[-1]))
                        nc.tensor.matmul(
                            psc[:], v_sb[:, kt, hh * 128:hh * 128 + 128],
                            pr[:], start=(kt == live[0]), stop=(kt == live[-1]))
                    rc = ap_.tile([128, 512], F32, name="rc", tag="rc")
                    nc.vector.reciprocal(rc[:], pss[:])
                    nc.vector.tensor_mul(ctx_sb[:, hh, t0:t0 + 512],
                                         psc[:], rc[:])

        # ===== phase C: dense (routed, bf16) -> feature-split RS =====
        with ExitStack() as pC:
            dwp = pC.enter_context(tc.tile_pool(name="dwp", bufs=1))
            dps = pC.enter_context(tc.tile_pool(name="dps", bufs=2, space="PSUM"))
            dop = pC.enter_context(tc.tile_pool(name="dop", bufs=4))
            dwts = []
            for ex, wsrc in ((0, wd0), (1, wd1)):
                dwt = dwp.tile([128, 2, H], BF16, name=f"dw{ex}", tag=f"dw{ex}")
                nc.sync.dma_start(dwt[:], r128(wsrc.ap()))
                dwts.append(dwt)
            # dram view [p, half, slot*mt, n]: row = half*4096 + c*128 + p
            rb = lambda t: t[:].rearrange("(h c p) n -> p h c n", h=2, p=128)
            bnc_a = rb(bounce_a)
            bnc_b = rb(bounce_b)
            for half, bnc in ((0, bnc_a), (1, bnc_b)):
                for ci in range(4):
                    t0 = ci * 512
                    sg = _segs(t0, t0 + 512, b0, b1, b2)
                    live = [x for x in sg if x[2]]
                    # staging tile [128, 2(half), 8(mt), 256]
                    big = dop.tile([128, 2, 8, 256], BF16, name="dot", tag="dot")
                    for mt in range(8 * half, 8 * half + 8):
                        ot = big[:, :, mt - 8 * half, :]
                        if live:
                            need = sorted({x for _, _, ex in live for x in ex})
                            pss_ = {}
                            for x in need:
                                ps = dps.tile([128, 512], F32, name=f"dpst{x}",
                                              tag=f"dpst{x}")
                                for kc in range(2):
                                    nc.tensor.matmul(
                                        ps[:],
                                        dwts[x][:, kc, mt * 128:mt * 128 + 128],
                                        ctx_sb[:, kc, t0:t0 + 512],
                                        start=(kc == 0), stop=(kc == 1))
                                pss_[x] = ps
                            for s, e, ex in sg:
                                so, eo = s - t0, e - t0
                                dsl = ot[:, so // 256:(eo + 255) // 256, :]
                                if so // 256 == (eo - 1) // 256:
                                    dsl = ot[:, so // 256, so % 256:
                                             so % 256 + (eo - so)]
                                else:
                                    dsl = None
                                if dsl is None:
                                    # crosses the 256 boundary: do per-half
                                    parts = [(so, min(eo, 256)), (max(so, 256), eo)]
                                    parts = [(a, b) for a, b in parts if a < b]
                                else:
                                    parts = [(so, eo)]
                                for a, b in parts:
                                    dv = ot[:, a // 256, a % 256:a % 256 + (b - a)]
                                    if len(ex) == 2:
                                        nc.vector.tensor_add(dv, pss_[0][:, a:b],
                                                             pss_[1][:, a:b])
                                    elif ex:
                                        nc.vector.tensor_copy(dv,
                                                              pss_[ex[0]][:, a:b])
                                    else:
                                        nc.vector.memset(dv, 0.0)
                        else:
                            nc.vector.memset(ot[:], 0.0)
                    for hx in range(2):
                        nc.sync.dma_start(bnc[:, hx, ci * 8:ci * 8 + 8, :],
                                          big[:, hx, :, :])
                if half == 0:
                    nc.gpsimd.collective_compute(
                        "ReduceScatter", mybir.AluOpType.add,
                        replica_groups=[list(range(NC_))],
                        ins=[bounce_a.opt()], outs=[rs_a.opt()])
                else:
                    nc.gpsimd.collective_compute(
                        "ReduceScatter", mybir.AluOpType.add,
                        replica_groups=[list(range(NC_))],
                        ins=[bounce_b.opt()], outs=[rs_b.opt()])
        pABC.close()

        pDP = top.enter_context(ExitStack())
        dp = pDP.enter_context(tc.tile_pool(name="dp", bufs=1))
        h1_sb = dp.tile([128, 16, 256], F32)
        pCC = top.enter_context(ExitStack())
        cctxp = pCC.enter_context(tc.tile_pool(name="cctxp", bufs=1))
        cctx_sb = cctxp.tile([128, 4, 512], BF16)
        pD2E = top.enter_context(ExitStack())
        d2 = pD2E.enter_context(tc.tile_pool(name="d2", bufs=1))
        # cross-attn K/V + wcq loads: kvagg ready mid-B, so these flow on the
        # sync queue during the RS window
        k_sb = d2.tile([128, 4, 2048], BF16)
        for hl in range(8):
            for src_ in range(4):
                nc.sync.dma_start(
                    k_sb[64 * (hl % 2):64 * (hl % 2) + 64, hl // 2,
                         512 * src_:512 * src_ + 512],
                    kvagg[1024 * src_ + 64 * hl:
                          1024 * src_ + 64 * hl + 64, 0:512])
        v_sb2 = d2.tile([128, 16, 520], BF16)
        for src_ in range(4):
            nc.sync.dma_start(
                v_sb2[:, 4 * src_:4 * src_ + 4, :],
                r128(kvagg[1024 * src_ + 512:1024 * src_ + 1024, :]))
        wcq_sb = d2.tile([128, 16, 512], BF16)
        nc.sync.dma_start(wcq_sb[:], r128(wcqh.ap()))

        # ===== phase D1: h1 = rs + resid, rmsnorm, pairwise h1n AG =====
        with ExitStack() as pD1:
            d1 = pD1.enter_context(tc.tile_pool(name="d1", bufs=1))
            d1ps = pD1.enter_context(tc.tile_pool(name="d1ps", bufs=1,
                                                  space="PSUM"))
            re_sb = d1.tile([128, 16, 256], F32)
            nc.sync.dma_start(re_sb[:], r128(resid.ap()))
            rsla = d1.tile([128, 8, 256], BF16)
            rslb = d1.tile([128, 8, 256], BF16)
            nc.sync.dma_start(rsla[:], r128(rs_a[:]))
            nc.sync.dma_start(rslb[:], r128(rs_b[:]))
            pss = d1ps.tile([128, 256], F32, name="psd", tag="psd")
            for kc in range(16):
                rsl = rsla if kc < 8 else rslb
                nc.vector.tensor_add(h1_sb[:, kc, :],
                                     rsl[:, kc % 8, :], re_sb[:, kc, :])
                sq = scrp.tile([128, 256], BF16, name="sqd", tag="sqd")
                nc.scalar.activation(sq[:], h1_sb[:, kc, :], SQ)
                nc.tensor.matmul(pss[:], ones_bf[:], sq[:],
                                 start=(kc == 0), stop=(kc == 15))
            rms1 = d1.tile([128, 256], F32)
            nc.scalar.activation(rms1[:], pss[:], SQRT,
                                 scale=1.0 / H, bias=eps_sb[:])
            rinv = d1.tile([128, 256], F32)
            nc.vector.reciprocal(rinv[:], rms1[:])
            h1n_sb = d1.tile([128, 16, 256], BF16)
            for kc in range(16):
                nc.vector.tensor_mul(h1n_sb[:, kc, :],
                                     h1_sb[:, kc, :], rinv[:])
            nc.sync.dma_start(r128(h1n_bnc[:]), h1n_sb[:])
        nc.gpsimd.collective_compute(
            "AllGather", mybir.AluOpType.bypass,
            replica_groups=[[0, 4], [1, 5], [2, 6], [3, 7]],
            ins=[h1n_bnc.opt()], outs=[h1n_pair.opt()])

        # ===== phase D2: cross-attn, 8 heads x 512 tokens =====
        with ExitStack() as pD2:
            h1n5 = d2.tile([128, 16, 512], BF16)
            for r in range(2):
                nc.sync.dma_start(h1n5[:, :, 256 * r:256 * r + 256],
                                  r128(h1n_pair[H * r:H * (r + 1), :]))
            cq_sb = d2.tile([128, 4, 512], BF16)
            d2ps = pD2.enter_context(tc.tile_pool(name="d2ps", bufs=2,
                                                  space="PSUM"))
            for sl in range(4):
                ps = d2ps.tile([128, 512], F32, name="cqp", tag="cqp")
                for kc in range(16):
                    nc.tensor.matmul(ps[:],
                                     wcq_sb[:, kc, sl * 128:sl * 128 + 128],
                                     h1n5[:, kc, :],
                                     start=(kc == 0), stop=(kc == 15))
                nc.vector.tensor_copy(cq_sb[:, sl, :], ps[:])
            with ExitStack() as pD3:
                cap = pD3.enter_context(tc.tile_pool(name="cap", bufs=3))
                caps = pD3.enter_context(tc.tile_pool(name="caps", bufs=2,
                                                      space="PSUM"))
                cacc = pD3.enter_context(tc.tile_pool(name="cacc", bufs=1,
                                                      space="PSUM"))
                for wave in range(2):
                    psc = [cacc.tile([65, 512], F32, name=f"cps{j}", tag=f"cps{j}")
                           for j in range(4)]
                    for kt in range(16):
                        for j in range(4):
                            hl = 4 * wave + j
                            sc = caps.tile([128, 512], F32, name="csc", tag="csc")
                            nc.tensor.matmul(
                                sc[:],
                                k_sb[64 * (hl % 2):64 * (hl % 2) + 64, hl // 2,
                                     kt * 128:kt * 128 + 128],
                                cq_sb[64 * (hl % 2):64 * (hl % 2) + 64,
                                      hl // 2, :],
                                start=True, stop=True)
                            pr = cap.tile([128, 512], BF16, name="cpr", tag="cpr")
                            nc.scalar.activation(pr[:], sc[:], EXP, scale=CSC)
                            nc.tensor.matmul(
                                psc[j][:],
                                v_sb2[:, kt, 65 * hl:65 * hl + 65],
                                pr[:], start=(kt == 0), stop=(kt == 15))
                    for j in range(4):
                        hl = 4 * wave + j
                        dnf = cap.tile([1, 512], F32, name="dnf", tag="dnf")
                        nc.vector.tensor_copy(dnf[:], psc[j][64:65, :])
                        rcf = cap.tile([1, 512], F32, name="rcf", tag="rcf")
                        nc.vector.reciprocal(rcf[:], dnf[:])
                        rcb = cap.tile([1, 512], BF16, name="rcb", tag="rcb")
                        nc.vector.tensor_copy(rcb[:], rcf[:])
                        dnb = caps.tile([64, 512], F32, name="dnb", tag="csc")
                        nc.tensor.matmul(dnb[:], ones_bf[0:1, 0:64], rcb[:],
                                         start=True, stop=True)
                        rbf = cap.tile([64, 512], F32, name="rbf", tag="rbf")
                        nc.vector.tensor_copy(rbf[:], dnb[:])
                        po = 64 * (hl % 2)
                        nc.vector.tensor_mul(cctx_sb[po:po + 64, hl // 2, :],
                                             psc[j][0:64, :], rbf[:])

        pD2E.close()
        # ===== phase D2b: cdense half-contraction -> pairwise RS =====
        with ExitStack() as pD4:
            d4 = pD4.enter_context(tc.tile_pool(name="d4", bufs=1))
            d4o = pD4.enter_context(tc.tile_pool(name="d4o", bufs=4))
            wcd_sb = d4.tile([128, 4, H], BF16)
            nc.sync.dma_start(wcd_sb[:], r128(wcdh.ap()))
            d4ps = pD4.enter_context(tc.tile_pool(name="d4ps", bufs=2,
                                                  space="PSUM"))
            cdp_t = cdpart[:].rearrange("(h c p) n -> p h c n", h=2, p=128)
            for mg in range(2):
                big = d4o.tile([128, 2, 8, 256], BF16, name="cdo", tag="cdo")
                for mj in range(8):
                    mt = mg * 8 + mj
                    ps = d4ps.tile([128, 512], F32, name="cdp", tag="cdp")
                    for kc in range(4):
                        nc.tensor.matmul(ps[:],
                                         wcd_sb[:, kc, mt * 128:mt * 128 + 128],
                                         cctx_sb[:, kc, :],
                                         start=(kc == 0), stop=(kc == 3))
                    nc.vector.tensor_copy(big[:, 0, mj, :], ps[:, 0:256])
                    nc.vector.tensor_copy(big[:, 1, mj, :], ps[:, 256:512])
                for hx in range(2):
                    nc.sync.dma_start(cdp_t[:, hx, mg * 8:mg * 8 + 8, :],
                                      big[:, hx, :, :])
        pCC.close()
        nc.gpsimd.collective_compute(
            "ReduceScatter", mybir.AluOpType.add,
            replica_groups=[[0, 4], [1, 5], [2, 6], [3, 7]],
            ins=[cdpart.opt()], outs=[cdsum.opt()])

        # ===== phase D3: h2 = h1 + cdsum, rmsnorm, split AG =====
        with ExitStack() as pD5:
            d5 = pD5.enter_context(tc.tile_pool(name="d5", bufs=1))
            cds_sb = d5.tile([128, 16, 256], BF16)
            nc.sync.dma_start(cds_sb[:], r128(cdsum[:]))
            h2_sb = d5.tile([128, 16, 256], F32)
            d5ps = pD5.enter_context(tc.tile_pool(name="d5ps", bufs=1,
                                                  space="PSUM"))
            pss2 = d5ps.tile([128, 256], F32, name="psd2", tag="psd2")
            for kc in range(16):
                nc.vector.tensor_add(h2_sb[:, kc, :], cds_sb[:, kc, :],
                                     h1_sb[:, kc, :])
                sq = scrp.tile([128, 256], BF16, name="sqd2", tag="sqd")
                nc.scalar.activation(sq[:], h2_sb[:, kc, :], SQ)
                nc.tensor.matmul(pss2[:], ones_bf[:], sq[:],
                                 start=(kc == 0), stop=(kc == 15))
            nc.sync.dma_start(r128(h2out.ap()), h2_sb[:])
            rms2 = d5.tile([128, 256], F32)
            nc.scalar.activation(rms2[:], pss2[:], SQRT,
                                 scale=1.0 / H, bias=eps_sb[:])
            rinv2 = d5.tile([128, 256], F32)
            nc.vector.reciprocal(rinv2[:], rms2[:])
            h2n_sb = d5.tile([128, 16, 256], BF16)
            for kc in range(16):
                nc.vector.tensor_mul(h2n_sb[:, kc, :],
                                     h2_sb[:, kc, :], rinv2[:])
            nc.sync.dma_start(r128(h2n_bnc[:]), h2n_sb[:])
        pDP.close()
        # MLP expert-0 weight prefetch: no deps, so these flow on the sync
        # queue during pairRS/D3/AG. Expert-1 tiles reuse the same tags, so
        # their DMAs are issued inside phase F (a tag-reuse wait here would
        # stall the sync queue pre-AG).
        fw = top.enter_context(tc.tile_pool(name="fw", bufs=1))

        def load_mlp_w(ex, gsrc, dsrc):
            dn_t = fw.tile([128, 6, H], BF16, name=f"dn{ex}", tag="dn")
            nc.sync.dma_start(dn_t[:], r128(dsrc.ap()))
            gws = []
            for pi in range(6):
                gw = 128 if pi < 5 else 48
                gwt = fw.tile([128, 16, 256], BF16,
                              name=f"guw{ex}{pi}", tag=f"guw{pi}")
                nc.sync.dma_start(
                    gwt[:, :, :2 * gw],
                    r128(gsrc.ap()[:, pi * 256:pi * 256 + 2 * gw]))
                gws.append(gwt)
            return dn_t, gws

        dn_ts = {}
        gw_ts = {}
        dn_ts[0], gw_ts[0] = load_mlp_w(0, wgu0, wdn0)
        nc.gpsimd.collective_compute(
            "AllGather", mybir.AluOpType.bypass,
            replica_groups=[list(range(NC_))],
            ins=[h2n_bnc.opt()], outs=[h2n_all.opt()])

        # ===== phase F: MLP (routed by expert ranges, bf16) =====
        with ExitStack() as pF:
            fp = pF.enter_context(tc.tile_pool(name="fp", bufs=1))
            hn_t = [fp.tile([128, 16, 512], BF16, name=f"hn{ci}")
                    for ci in range(4)]
            for ci in range(4):
                for j, r in enumerate((ci, ci + 4)):
                    nc.sync.dma_start(
                        hn_t[ci][:, :, 256 * j:256 * j + 256],
                        r128(h2n_all[r * H:(r + 1) * H, :]))
            fps = pF.enter_context(tc.tile_pool(name="fps", bufs=2, space="PSUM"))
            fpd = pF.enter_context(tc.tile_pool(name="fpd", bufs=2, space="PSUM"))
            fac = pF.enter_context(tc.tile_pool(name="fac", bufs=2))
            fout = pF.enter_context(tc.tile_pool(name="fout", bufs=4))
            for ex, (lo, hi) in ((0, (0, b1)), (1, (b1, S))):
                if ex not in dn_ts:
                    dn_ts[ex], gw_ts[ex] = load_mlp_w(ex, (wgu0, wgu1)[ex],
                                                      (wdn0, wdn1)[ex])
                dn_t = dn_ts[ex]
                gwts = gw_ts[ex]
                for a0 in range(0, S, 512):
                    c0, c1 = max(a0, lo), min(a0 + 512, hi)
                    if c0 >= c1:
                        continue
                    t0_, W = a0, 512
                    eo, ew = c0 - a0, c1 - c0
                    act = fac.tile([128, 6, 512], BF16, name="act", tag="act")
                    for pi in range(6):
                        gw = 128 if pi < 5 else 48
                        gwt = gwts[pi]
                        pg = fps.tile([128, 512], F32, name="pg", tag="pg")
                        pu = fps.tile([128, 512], F32, name="pu", tag="pu")
                        hnc = hn_t[t0_ // 512]
                        for kc in range(16):
                            nc.tensor.matmul(pg[:gw, :W], gwt[:, kc, :gw],
                                             hnc[:, kc, :],
                                             start=(kc == 0), stop=(kc == 15))
                            nc.tensor.matmul(pu[:gw, :W], gwt[:, kc, gw:2 * gw],
                                             hnc[:, kc, :],
                                             start=(kc == 0), stop=(kc == 15))
                        gs = scrp.tile([128, 512], F32, name="gs", tag="gs")
                        nc.scalar.activation(gs[:gw, :W], pg[:gw, :W], SILU)
                        nc.vector.tensor_mul(act[:gw, pi, :W],
                                             gs[:gw, :W], pu[:gw, :W])
                    for mt in range(16):
                        pd = fpd.tile([128, 512], F32, name="pd", tag="pd")
                        for pi in range(6):
                            kw = 128 if pi < 5 else 48
                            nc.tensor.matmul(
                                pd[:, :W],
                                dn_t[:kw, pi, mt * 128:mt * 128 + 128],
                                act[:kw, pi, :W],
                                start=(pi == 0), stop=(pi == 5))
                        ot = fout.tile([128, 512], BF16, name="fot", tag="fot")
                        nc.vector.tensor_copy(ot[:, eo:eo + ew], pd[:, eo:eo + ew])
                        nc.sync.dma_start(
                            y.ap()[mt * 128:mt * 128 + 128, c0:c1],
                            ot[:, eo:eo + ew])
    nc.compile()
    return nc


_CACHE = {}


def kernel(**inputs):
    import ml_dtypes
    vm = np.asarray(inputs["vision_token_ids"]).astype(bool)
    lm = np.asarray(inputs["language_token_ids"]).astype(bool)
    g0 = np.where(vm & ~lm)[0]; g1 = np.where(vm & lm)[0]
    g2 = np.where(~vm & lm)[0]; g3 = np.where(~vm & ~lm)[0]
    perm = np.concatenate([g0, g1, g2, g3])
    b0 = len(g0); b1 = b0 + len(g1); b2 = b1 + len(g2)

    f32 = lambda x: np.ascontiguousarray(np.asarray(x, np.float32))
    bf = lambda x: np.ascontiguousarray(np.asarray(x).astype(ml_dtypes.bfloat16))
    pos = np.asarray(inputs["positions"]).astype(np.float32)
    half = HD // 2
    inv_freq = 1.0 / (ROPE_BASE ** (np.arange(half, dtype=np.float32) / half))
    fr = pos[:, None] * inv_freq[None, :]
    cos2 = np.concatenate([np.cos(fr)] * 2, 1).T[:, perm]
    sin2 = np.concatenate([np.sin(fr)] * 2, 1).T[:, perm]
    rot = np.zeros((HD, HD), np.float32)
    rot[np.arange(half), np.arange(half) + half] = -1.0
    rot[np.arange(half) + half, np.arange(half)] = 1.0
    op = np.asarray(inputs["positions"])[perm].astype(np.int64)

    # block classification: rows=keys [128kt,+128), cols=queries [512qc,+512)
    blocks = {}
    mix_tiles = []
    for qc in range(4):
        opq = op[512 * qc:512 * qc + 512]
        for kt in range(16):
            opk = op[128 * kt:128 * kt + 128]
            if opq.min() >= opk.max():
                blocks[(qc, kt)] = ('F', -1)
            elif opq.max() < opk.min():
                pass  # skip
            else:
                blocks[(qc, kt)] = ('M', len(mix_tiles))
                mix_tiles.append(
                    np.where(opq[None, :] >= opk[:, None], 0.0, -30000.0))
    maskmix = (np.concatenate(mix_tiles, 0) if mix_tiles
               else np.zeros((128, 512), np.float32))

    wln_in = f32(inputs["w_ln_in"])[:, None]
    wln_pa = f32(inputs["w_ln_post_attn"])[:, None]
    wln_pc = f32(inputs["w_ln_post_cross"])[:, None]
    wqkv = [f32(inputs["w_vis_qkv"]) * wln_in, f32(inputs["w_lang_qkv"]) * wln_in]
    wd = [f32(inputs["w_vis_dense"]), f32(inputs["w_lang_dense"])]
    wgu = [f32(inputs["w_vis_gate_up"]) * wln_pc,
           f32(inputs["w_lang_gate_up"]) * wln_pc]
    wdn = [f32(inputs["w_vis_down"]), f32(inputs["w_lang_down"])]
    wkvf = f32(inputs["w_cross_kv"])
    wcqf = f32(inputs["w_cross_q"]) * wln_pa
    wcdf = f32(inputs["w_cross_dense"])
    encTf = f32(inputs["encoder_embeds"]).T
    hTp = f32(inputs["hidden_states"]).T[:, perm].copy()

    def interleave(w):  # w [H, 2*IS] = [gate | up]
        cols = []
        for i in range(5):
            cols.append(w[:, 128 * i:128 * i + 128])
            cols.append(w[:, IS + 128 * i:IS + 128 * i + 128])
        cols.append(w[:, 640:IS]); cols.append(w[:, IS + 640:2 * IS])
        return np.ascontiguousarray(np.concatenate(cols, 1))

    bsig = tuple(sorted((k, v[0]) for k, v in blocks.items()))
    key = (b0, b1, b2, bsig)
    if key not in _CACHE:
        _CACHE.clear()
        _CACHE[key] = build_kernel(b0, b1, b2, blocks)
    nc = _CACHE[key]

    in_maps = []
    tslices = []
    for c in range(NC_):
        hh, g = c // 4, c % 4
        ts = slice(512 * g + 256 * hh, 512 * g + 256 * hh + 256)
        tslices.append(ts)
        qs = slice(256 * c, 256 * c + 256)
        m = dict(
            hT=bf(hTp),
            wqkv0=bf(np.concatenate([wqkv[0][:, qs], wqkv[0][:, H:][:, qs],
                                     wqkv[0][:, 2 * H:][:, qs]], 1)),
            wqkv1=bf(np.concatenate([wqkv[1][:, qs], wqkv[1][:, H:][:, qs],
                                     wqkv[1][:, 2 * H:][:, qs]], 1)),
            wd0=bf(wd[0][qs]), wd1=bf(wd[1][qs]),
            cos2=bf(cos2), sin2=bf(sin2), rotT=bf(rot.T),
            onesb=np.ones((128, 128), ml_dtypes.bfloat16),
            maskmix=bf(maskmix),
            resid=hTp[:, ts].copy(),
            encTs=bf(encTf[:, 512 * g:512 * g + 512]),
            wkh=bf(wkvf[:, :CC][:, 512 * hh:512 * hh + 512]),
            wvh=bf(wkvf[:, CC:][:, 512 * hh:512 * hh + 512]),
            wcqh=bf(wcqf[:, 512 * hh:512 * hh + 512]),
            wcdh=bf(wcdf[512 * hh:512 * hh + 512, :]),
            wgu0=bf(interleave(np.concatenate(
                [wgu[0][:, IS * c:IS * c + IS],
                 wgu[0][:, I + IS * c:I + IS * c + IS]], 1))),
            wgu1=bf(interleave(np.concatenate(
                [wgu[1][:, IS * c:IS * c + IS],
                 wgu[1][:, I + IS * c:I + IS * c + IS]], 1))),
            wdn0=bf(np.concatenate([wdn[0][IS * c:IS * c + IS],
                                    np.zeros((ISP - IS, H), np.float32)], 0)),
            wdn1=bf(np.concatenate([wdn[1][IS * c:IS * c + IS],
                                    np.zeros((ISP - IS, H), np.float32)], 0)),
        )
        in_maps.append(m)

    trace = bool(int(os.environ.get("KTRACE", "0")))
    res = run_bass_kernel_spmd(nc, in_maps, core_ids=list(range(NC_)),
                               trace=trace,
                               tmpdir=os.environ.get("KTRACE_DIR") or None)
    kernel.last_exec_ns = res.exec_time_ns
    kernel.last_trace = res.instructions_and_trace
    kernel.last_results = res.results
    kernel.last_tslices = tslices
    tot = res.results[0]["y"].astype(np.float64)
    for c in range(1, NC_):
        tot += res.results[c]["y"]
    for c in range(NC_):
        tot[:, tslices[c]] += res.results[c]["h2out"]
    out = np.empty((S, H), np.float32)
    out[perm, :] = tot.T.astype(np.float32)
    return out
